# revision 21
# baseline (speedup 1.0000x reference)
"""Trainium2 Bass kernel for nn_CustomTransformer_64570538328578.

Encoder-decoder transformer: V=32000, D=512, H=8, L=6+6, DFF=2048, B=4, S=512.

Sharding: 8 cores = 4 batch pairs x 2 vocab halves.  Core c handles batch
element c//2 (full encoder+decoder stack, duplicated within the pair) and
computes logits for vocab half c%2 of the output projection.  No on-device
collectives needed.

All matmul data is bf16 (PSUM accumulation fp32).  Attention is computed in
transposed-score layout: ST[k,q] = (K^T)^T-by-Q products per k-tile, exp on
ACT, and the softmax denominator comes for free from a ones-column appended
to the V stationary (65-row ctx matmul).  The per-query reciprocal is
broadcast across partitions with a rank-1 fp32r matmul, so no per-head
transposes of the attention matrix are needed at all.

Layouts on device (per core):
  - canonical activations x: [S, D] as 4 tiles [128, 512] (token-partition)
  - transposed activations xT: [D, S] as 4 tiles [128, 512] (dim-partition)
  - per-head QT/KT: [DK, S] packed 2 heads/tile -> 4 tiles [128, 512]
  - V+ones: [S, 8*65] 4 tiles (per-head 64 dims + ones col side by side)
  - scoresT per (head, k-tile): PSUM [128, <=512]; causal diag mask added
    via an ident x tril(-1e9) matmul into the first 128 columns.
"""

import math
import sys

import ml_dtypes
import numpy as np

if "/opt/trn_rl_repo" not in sys.path:
    sys.path.insert(0, "/opt/trn_rl_repo")

import concourse.bass as bass
import concourse.tile as tile
from concourse import bacc
from concourse import mybir
from concourse.bass import ds, ts
from concourse.bass_utils import run_bass_kernel_spmd

FP = mybir.dt.float32
F32R = mybir.dt.float32r
S = 512
D = 512
H = 8
DK = 64
DFF = 2048
LE = 6
LD = 6
V = 32000
NCORES = 8
VH = V // 2        # vocab half per core
TT = 4             # token tiles (S / 128)
DC = 4             # D chunks of 128
G = 4              # head-pair groups (2 heads of 64 dims -> 128 partitions)
FTL = 16           # dff tiles of 128
NEG = -1.0e9
AF = mybir.ActivationFunctionType
FR = mybir.dt.bfloat16   # matmul/activation storage dtype
BFNP = ml_dtypes.bfloat16


def _f32r(ap):
    return ap.bitcast(F32R)


def _view3(ap, groups, gstride, inner, inner_off=0):
    """[128, x] AP -> [128, groups, inner] view with group stride gstride."""
    a = ap
    return bass.AP(
        tensor=a.tensor,
        offset=a.offset + inner_off,
        ap=[a.ap[0], [gstride, groups], [1, inner]],
    )


class _StopTrace(Exception):
    pass


def _build_program(flags, debug_stage=None):
    """Build the single SPMD Bass program (same for all cores)."""
    nc = bacc.Bacc(None)

    # ---- DRAM parameters ----------------------------------------------------
    x0e = nc.declare_dram_parameter("x0e", [S, D], FR, isOutput=False)
    x0d = nc.declare_dram_parameter("x0d", [S, D], FR, isOutput=False)
    we_attn = nc.declare_dram_parameter("we_attn", [LE, 4, D, D], FR, isOutput=False)
    we_f1 = nc.declare_dram_parameter("we_f1", [LE, D, DFF], FR, isOutput=False)
    we_f2 = nc.declare_dram_parameter("we_f2", [LE, DFF, D], FR, isOutput=False)
    wd_sa = nc.declare_dram_parameter("wd_sa", [LD, 4, D, D], FR, isOutput=False)
    wd_ca = nc.declare_dram_parameter("wd_ca", [LD, 4, D, D], FR, isOutput=False)
    wd_f1 = nc.declare_dram_parameter("wd_f1", [LD, D, DFF], FR, isOutput=False)
    wd_f2 = nc.declare_dram_parameter("wd_f2", [LD, DFF, D], FR, isOutput=False)
    wout = nc.declare_dram_parameter("wout", [D, VH], FR, isOutput=False)
    cident = nc.declare_dram_parameter("cident", [128, 128], FR, isOutput=False)
    ctri = nc.declare_dram_parameter("ctri", [128, 128], FR, isOutput=False)
    cones = nc.declare_dram_parameter("cones", [1, 64], F32R, isOutput=False)
    logits = nc.declare_dram_parameter("logits", [S, VH], FP, isOutput=True)
    dbg = None
    if debug_stage is not None:
        dbg = nc.declare_dram_parameter("dbg", [8, 128, 512], FP, isOutput=True)

    # optional (general-path) params, declared only when actually used
    if flags.get("attn_bias"):
        be_attn = nc.declare_dram_parameter("be_attn", [LE, 4, D], FP, isOutput=False)
        bd_sa = nc.declare_dram_parameter("bd_sa", [LD, 4, D], FP, isOutput=False)
        bd_ca = nc.declare_dram_parameter("bd_ca", [LD, 4, D], FP, isOutput=False)
    if flags.get("ffn_bias"):
        be_f1 = nc.declare_dram_parameter("be_f1", [LE, DFF], FP, isOutput=False)
        be_f2 = nc.declare_dram_parameter("be_f2", [LE, D], FP, isOutput=False)
        bd_f1 = nc.declare_dram_parameter("bd_f1", [LD, DFF], FP, isOutput=False)
        bd_f2 = nc.declare_dram_parameter("bd_f2", [LD, D], FP, isOutput=False)
    if flags.get("ln_affine"):
        eln_g = nc.declare_dram_parameter("eln_g", [LE, 2, D], FP, isOutput=False)
        eln_b = nc.declare_dram_parameter("eln_b", [LE, 2, D], FP, isOutput=False)
        dln_g = nc.declare_dram_parameter("dln_g", [LD, 3, D], FP, isOutput=False)
        dln_b = nc.declare_dram_parameter("dln_b", [LD, 3, D], FP, isOutput=False)
    if flags.get("out_bias"):
        bout = nc.declare_dram_parameter("bout", [VH], FP, isOutput=False)
    if flags.get("src_mask"):
        km_src = nc.declare_dram_parameter("km_src", [S], FP, isOutput=False)
    if flags.get("tgt_mask"):
        rm_tgt = nc.declare_dram_parameter("rm_tgt", [S], FP, isOutput=False)
        cm_tgt = nc.declare_dram_parameter("cm_tgt", [S, S], FP, isOutput=False)

    from contextlib import ExitStack

    with tile.TileContext(nc) as tc, ExitStack() as stk:
      try:
        wpool = stk.enter_context(tc.tile_pool(name="w", bufs=64))
        apool = stk.enter_context(tc.tile_pool(name="acts", bufs=2))
        hpool = stk.enter_context(tc.tile_pool(name="h", bufs=16))
        cpool = stk.enter_context(tc.tile_pool(name="consts", bufs=1))
        spool = stk.enter_context(tc.tile_pool(name="small", bufs=12))
        pspool = stk.enter_context(tc.tile_pool(name="ps", bufs=2, space="PSUM"))
        pcpool = stk.enter_context(tc.tile_pool(name="pc", bufs=3, space="PSUM"))
        scpool = stk.enter_context(tc.tile_pool(name="sc", bufs=3, space="PSUM"))
        if True:

            # ---- constants --------------------------------------------------
            ident = cpool.tile([128, 128], FR)
            nc.sync.dma_start(out=ident, in_=cident[:, :])
            tri = cpool.tile([128, 128], FR)
            nc.sync.dma_start(out=tri, in_=ctri[:, :])
            eps = cpool.tile([128, 1], FP)
            nc.vector.memset(eps, 1e-5)
            ones64 = cpool.tile([1, 64], F32R)
            nc.sync.dma_start(out=ones64, in_=cones[:, :])

            bcast = None
            if flags.get("src_mask"):
                bcast = cpool.tile([128, S], FP)
                kma = km_src[:]
                nc.sync.dma_start(
                    out=bcast,
                    in_=bass.AP(
                        tensor=kma.tensor,
                        offset=kma.offset,
                        ap=[[0, 128]] + kma.ap,
                    ),
                )
            rmt = None
            if flags.get("tgt_mask"):
                rmt = cpool.tile([128, TT], FP)
                for t in range(TT):
                    nc.sync.dma_start(out=rmt[:, t : t + 1], in_=rm_tgt[ts(t, 128)])
                cmt = []
                for t in range(TT):
                    cm = cpool.tile([128, S], FP, tag="cmt")
                    nc.sync.dma_start(out=cm, in_=cm_tgt[ts(t, 128), :])
                    cmt.append(cm)

            junk = cpool.tile([128, 1], FP, tag="junk")
            nc.vector.memset(junk, 1.0)

            def act_warm(func):
                # dummy activation to pull the ACT function table in while
                # the PE is busy, so the real op doesn't pay the ~1.3us
                # table reload on the LN critical path
                j = spool.tile([128, 1], FP, tag="jk", bufs=2)
                nc.scalar.activation(out=j, in_=junk, func=func)

            def wtile(dram_ap, tag="w"):
                t_ = wpool.tile([128, 512], FR, tag=tag)
                rows = dram_ap.shape[-2]
                nc.sync.dma_start(out=t_[:rows, :], in_=dram_ap)
                return t_

            def bias_col(dram_ap):
                n = dram_ap.shape[-1]
                b = spool.tile([128, 1], FP, tag="bias")
                nc.sync.dma_start(out=b[:n, :], in_=dram_ap)
                return b[:n, :]

            def bcast_tile(dram_ap, n):
                """[n] dram vector -> [128, n] sbuf tile (partition broadcast)."""
                b = apool.tile([128, n], FP, tag="bc", bufs=2)
                nc.sync.dma_start(
                    out=b,
                    in_=bass.AP(
                        tensor=dram_ap.tensor,
                        offset=dram_ap.offset,
                        ap=[[0, 128]] + dram_ap.ap,
                    ),
                )
                return b

            def dump_and_stop(tiles):
                for j, t_ in enumerate(tiles[:8]):
                    stt = apool.tile([128, 512], FP, tag="ob", bufs=4)
                    pp, ff = t_.shape[-2], t_.shape[-1]
                    nc.any.tensor_copy(out=stt[:pp, :ff], in_=t_)
                    nc.sync.dma_start(out=dbg[j, :pp, :ff], in_=stt[:pp, :ff])
                raise _StopTrace

            # ---- transpose helper: [S,D] tiles -> [D,S] tiles ---------------
            def transpose_sd(sd_tiles, out_tag):
                ds_tiles = []
                for c in range(DC):
                    trp = scpool.tile([128, 512], FR, tag="scp")
                    for t in range(TT):
                        nc.tensor.transpose(
                            out=trp[:, ds(128 * t, 128)],
                            in_=sd_tiles[t][:, ds(128 * c, 128)],
                            identity=ident,
                        )
                    xt = apool.tile([128, 512], FR, tag=out_tag, bufs=8)
                    # ACT is idle during the LN phase; keep DVE free for stats
                    nc.scalar.copy(out=xt, in_=trp)
                    ds_tiles.append(xt)
                return ds_tiles

            # ---- layernorm (input z already includes the residual) ----------
            def ln_block(z_sd, xt_tag, g_ap=None, b_ap=None):
                new_sd = []
                for t in range(TT):
                    z = z_sd[t]
                    st6 = spool.tile([128, 6], FP, tag="st6")
                    nc.vector.bn_stats(out=st6, in_=z)
                    mv = spool.tile([128, 2], FP, tag="mv")
                    nc.vector.bn_aggr(out=mv, in_=st6)
                    sd_ = spool.tile([128, 1], FP, tag="sd")
                    nc.scalar.activation(out=sd_, in_=mv[:, 1:2], func=AF.Sqrt, bias=eps)
                    rr = spool.tile([128, 1], FP, tag="rr")
                    nc.vector.reciprocal(out=rr, in_=sd_)
                    xn = apool.tile([128, 512], FR, tag="xn", bufs=8)
                    nc.vector.tensor_scalar(
                        out=xn, in0=z, scalar1=mv[:, 0:1], scalar2=rr,
                        op0=mybir.AluOpType.subtract, op1=mybir.AluOpType.mult,
                    )
                    if g_ap is not None:
                        gt_ = bcast_tile(g_ap, 512)
                        nc.vector.tensor_mul(xn, xn, gt_)
                    if b_ap is not None:
                        bt_ = bcast_tile(b_ap, 512)
                        nc.vector.tensor_add(xn, xn, bt_)
                    new_sd.append(xn)
                return new_sd, transpose_sd(new_sd, xt_tag)

            def residual(x_t, ps, badd_ap=None):
                """z = x_t + ps (+ badd broadcast); returns SBUF bf16 tile."""
                z = apool.tile([128, 512], FR, tag="z", bufs=4)
                nc.vector.tensor_add(z, x_t, ps)
                if badd_ap is not None:
                    bt = bcast_tile(badd_ap, 512)
                    nc.vector.tensor_add(z, z, bt)
                return z

            # ---- multi-head attention, transposed-score layout --------------
            mha_ctr = [0]

            def mha(x_sd, xq_ds, xkv_ds, w_ap, causal, badd_ap=None):
                """Fast path (no padding masks, no attn biases).
                Returns 4 SBUF z tiles (attn output + residual)."""
                midx = mha_ctr[0]; mha_ctr[0] += 1
                act_warm(AF.Exp)
                wq = [wtile(w_ap[0, ts(c, 128), :]) for c in range(DC)]
                wk = [wtile(w_ap[1, ts(c, 128), :]) for c in range(DC)]
                wv = [wtile(w_ap[2, ts(c, 128), :]) for c in range(DC)]
                wo = [wtile(w_ap[3, ds(64 * h, 64), :]) for h in range(H)]

                QT, KT = [], []
                for g in range(G):
                    psq = pspool.tile([128, 512], FP, tag="mm")
                    for c in range(DC):
                        nc.tensor.matmul(
                            out=psq, lhsT=wq[c][:, ts(g, 128)], rhs=xq_ds[c],
                            start=(c == 0), stop=(c == DC - 1),
                        )
                    qt = apool.tile([128, 512], FR, tag="qt", bufs=5)
                    nc.vector.tensor_copy(out=qt, in_=psq)
                    QT.append(qt)
                    psk = pspool.tile([128, 512], FP, tag="mm")
                    for c in range(DC):
                        nc.tensor.matmul(
                            out=psk, lhsT=wk[c][:, ts(g, 128)], rhs=xkv_ds[c],
                            start=(c == 0), stop=(c == DC - 1),
                        )
                    kt = apool.tile([128, 512], FR, tag="kt", bufs=5)
                    nc.vector.tensor_copy(out=kt, in_=psk)
                    KT.append(kt)

                # V with a ones column appended per head: [128 tok, 8*65]
                VO = []
                for t in range(TT):
                    psv = pspool.tile([128, 512], FP, tag="mm")
                    for c in range(DC):
                        nc.tensor.matmul(
                            out=psv, lhsT=xkv_ds[c][:, ts(t, 128)], rhs=wv[c],
                            start=(c == 0), stop=(c == DC - 1),
                        )
                    vvo = apool.tile([128, 8 * 65], FR, tag="vvo", bufs=5)
                    va = vvo[:, :]
                    nc.vector.memset(
                        bass.AP(tensor=va.tensor, offset=va.offset + 64,
                                ap=[va.ap[0], [65, 8], [1, 1]]), 1.0)
                    nc.vector.tensor_copy(
                        out=_view3(va, 8, 65, 64),
                        in_=_view3(psv[:, :], 8, 64, 64),
                    )
                    VO.append(vvo)
                if debug_stage == f"mha{midx}.qkv":
                    dump_and_stop(QT + KT)
                if debug_stage == f"mha{midx}.v":
                    dump_and_stop(VO)

                ctxT = []
                for g in range(G):
                    pscs = []
                    for lh in range(2):
                        h = 2 * g + lh
                        qsl = slice(64 * lh, 64 * lh + 64)
                        ES = []
                        for kt in range(TT):
                            q0 = 128 * kt if causal else 0
                            W = 512 - q0
                            scp = scpool.tile([128, 512], FP, tag="scp")
                            nc.tensor.matmul(
                                out=scp[:, ds(q0, W)],
                                lhsT=KT[g][qsl, ts(kt, 128)],
                                rhs=QT[g][qsl, ds(q0, W)],
                                start=True, stop=True,
                            )
                            if causal:
                                # add tril(-1e9, -1) to the diagonal block on
                                # DVE (frees a PE matmul+ldweights per k-tile)
                                nc.vector.tensor_add(
                                    scp[:, ds(q0, 128)], scp[:, ds(q0, 128)], tri)
                            es = apool.tile([128, 512], FR, tag="exp", bufs=8)
                            nc.scalar.activation(out=es[:, ds(q0, W)],
                                                 in_=scp[:, ds(q0, W)], func=AF.Exp)
                            ES.append(es)
                        psc = pcpool.tile([128, 512], FP, tag="psc")
                        for kt in range(TT):
                            q0 = 128 * kt if causal else 0
                            W = 512 - q0
                            nc.tensor.matmul(
                                out=psc[:65, ds(q0, W)],
                                lhsT=VO[kt][:, ds(65 * h, 65)],
                                rhs=ES[kt][:, ds(q0, W)],
                                start=(kt == 0), stop=(kt == TT - 1),
                            )
                        # softmax denominator: reciprocal of the ones-row, then
                        # partition-broadcast on the (idle) gpsimd engine
                        rec = spool.tile([1, 512], FR, tag="rec", bufs=4)
                        with nc.allow_low_precision(reason="bf16 softmax recip"):
                            nc.vector.reciprocal(out=rec, in_=psc[64:65, :])
                        rbs = apool.tile([64, 512], FR, tag="rbs", bufs=4)
                        nc.gpsimd.partition_broadcast(rbs[:, :], rec[:, :])
                        ct = apool.tile([64, 512], FR, tag="ctxh", bufs=10)
                        nc.vector.tensor_mul(ct, psc[:64, :], rbs)
                        ctxT.append(ct)

                if debug_stage == f"mha{midx}.ctx":
                    dump_and_stop(ctxT)
                z_sd = []
                for t in range(TT):
                    pso = pspool.tile([128, 512], FP, tag="mm")
                    for h in range(H):
                        nc.tensor.matmul(
                            out=pso, lhsT=ctxT[h][:, ts(t, 128)],
                            rhs=wo[h][:64, :],
                            start=(h == 0), stop=(h == H - 1),
                        )
                    z_sd.append(residual(x_sd[t], pso, badd_ap))
                if debug_stage == f"mha{midx}.out":
                    dump_and_stop(z_sd)
                return z_sd

            # ---- masked multi-head attention (general path) -----------------
            def mha_masked(x_sd, xq_ds, xkv_ds, w_ap, b_ap, causal, kmask,
                           badd_ap=None):
                midx = mha_ctr[0]; mha_ctr[0] += 1
                act_warm(AF.Exp)
                wq = [wtile(w_ap[0, ts(c, 128), :]) for c in range(DC)]
                wk = [wtile(w_ap[1, ts(c, 128), :]) for c in range(DC)]
                wv = [wtile(w_ap[2, ts(c, 128), :]) for c in range(DC)]
                wo = [wtile(w_ap[3, ds(64 * h, 64), :]) for h in range(H)]

                QT, KT = [], []
                for g in range(G):
                    psq = pspool.tile([128, 512], FP, tag="mm")
                    for c in range(DC):
                        nc.tensor.matmul(
                            out=psq, lhsT=wq[c][:, ts(g, 128)], rhs=xq_ds[c],
                            start=(c == 0), stop=(c == DC - 1),
                        )
                    qt = apool.tile([128, 512], FR, tag="qt", bufs=5)
                    if b_ap is not None:
                        nc.scalar.activation(out=qt, in_=psq, func=AF.Identity,
                                             bias=bias_col(b_ap[0, ts(g, 128)]))
                    else:
                        nc.scalar.copy(out=qt, in_=psq)
                    QT.append(qt)
                for g in range(G):
                    psk = pspool.tile([128, 512], FP, tag="mm")
                    for c in range(DC):
                        nc.tensor.matmul(
                            out=psk, lhsT=wk[c][:, ts(g, 128)], rhs=xkv_ds[c],
                            start=(c == 0), stop=(c == DC - 1),
                        )
                    kt = apool.tile([128, 512], FR, tag="kt", bufs=5)
                    if b_ap is not None:
                        nc.scalar.activation(out=kt, in_=psk, func=AF.Identity,
                                             bias=bias_col(b_ap[1, ts(g, 128)]))
                    else:
                        nc.scalar.copy(out=kt, in_=psk)
                    KT.append(kt)
                VV = []
                for t in range(TT):
                    psv = pspool.tile([128, 512], FP, tag="mm")
                    for c in range(DC):
                        nc.tensor.matmul(
                            out=psv, lhsT=xkv_ds[c][:, ts(t, 128)], rhs=wv[c],
                            start=(c == 0), stop=(c == DC - 1),
                        )
                    vv = apool.tile([128, 512], FR, tag="vvo", bufs=5)
                    if b_ap is not None:
                        bt = bcast_tile(b_ap[2], 512)
                        nc.vector.tensor_add(vv, psv, bt)
                    else:
                        nc.scalar.copy(out=vv, in_=psv)
                    VV.append(vv)

                ctxT = []
                for g in range(G):
                    for lh in range(2):
                        h = 2 * g + lh
                        psc = pcpool.tile([128, 512], FP, tag="psc")
                        qsl = slice(64 * lh, 64 * lh + 64)
                        att = []
                        for t in range(TT):
                            nch = TT if not causal else (t + 1)
                            W = 128 * nch
                            scp = scpool.tile([128, 512], FP, tag="scp")
                            nc.tensor.matmul(
                                out=scp[:, :W],
                                lhsT=QT[g][qsl, ts(t, 128)],
                                rhs=KT[g][qsl, :W],
                                start=True, stop=not causal,
                            )
                            if causal:
                                nc.tensor.matmul(
                                    out=scp[:, ds(128 * t, 128)],
                                    lhsT=tri, rhs=ident,
                                    start=False, stop=True,
                                )
                            if kmask is not None:
                                nc.vector.tensor_add(scp[:, :W], scp[:, :W], kmask[:, :W])
                            if causal and flags.get("tgt_mask"):
                                nc.vector.tensor_scalar_mul(scp[:, :W], scp[:, :W],
                                                            rmt[:, t : t + 1])
                                nc.vector.tensor_add(scp[:, :W], scp[:, :W], cmt[t][:, :W])
                            et = apool.tile([128, 512], FR, tag="exp", bufs=8)
                            ssum = spool.tile([128, 1], FP, tag="ssum")
                            nc.scalar.activation(out=et[:, :W], in_=scp[:, :W],
                                                 func=AF.Exp, accum_out=ssum)
                            rr = spool.tile([128, 1], FP, tag="srr")
                            nc.vector.reciprocal(out=rr, in_=ssum)
                            nc.vector.tensor_scalar_mul(et[:, :W], et[:, :W], rr)
                            att.append(et)
                        for c in range(TT):
                            t0 = c if causal else 0
                            wq_ = 512 - 128 * t0
                            trp = scpool.tile([128, 512], FR, tag="scp")
                            for t in range(t0, TT):
                                nc.tensor.transpose(
                                    out=trp[:, ds(128 * t, 128)],
                                    in_=att[t][:, ds(128 * c, 128)],
                                    identity=ident,
                                )
                            aT = apool.tile([128, 512], FR, tag="rbs", bufs=4)
                            nc.any.tensor_copy(out=aT[:, ds(128 * t0, wq_)],
                                               in_=trp[:, ds(128 * t0, wq_)])
                            nc.tensor.matmul(
                                out=psc[:64, ds(128 * t0, wq_)],
                                lhsT=VV[c][:, ds(64 * h, 64)],
                                rhs=aT[:, ds(128 * t0, wq_)],
                                start=(c == 0), stop=(c == TT - 1),
                            )
                        ct = apool.tile([64, 512], FR, tag="ctxh", bufs=10)
                        if b_ap is not None:
                            nc.scalar.activation(out=ct, in_=psc[:64, :],
                                                 func=AF.Identity,
                                                 bias=bias_col(b_ap[2, ds(64 * h, 64)]))
                        else:
                            nc.scalar.copy(out=ct, in_=psc[:64, :])
                        ctxT.append(ct)

                z_sd = []
                for t in range(TT):
                    pso = pspool.tile([128, 512], FP, tag="mm")
                    for h in range(H):
                        nc.tensor.matmul(
                            out=pso, lhsT=ctxT[h][:, ts(t, 128)],
                            rhs=wo[h][:64, :],
                            start=(h == 0), stop=(h == H - 1),
                        )
                    z_sd.append(residual(x_sd[t], pso, badd_ap))
                return z_sd

            def mha_any(x_sd, xq_ds, xkv_ds, w_ap, b_ap, causal, kmask,
                        badd_ap=None):
                use_masked = (b_ap is not None or kmask is not None
                              or (causal and flags.get("tgt_mask")))
                if use_masked:
                    return mha_masked(x_sd, xq_ds, xkv_ds, w_ap, b_ap, causal,
                                      kmask, badd_ap)
                return mha(x_sd, xq_ds, xkv_ds, w_ap, causal, badd_ap)

            # ---- FFN --------------------------------------------------------
            def ffn(x_sd, x_ds, w1_ap, w2_ap, b1_ap, b2_ap):
                w1 = [[wtile(w1_ap[ts(c, 128), ts(g2, 512)]) for g2 in range(4)]
                      for c in range(DC)]
                w2 = [wtile(w2_ap[ts(dt, 128), :]) for dt in range(FTL)]
                hT = []
                for dt in range(FTL):
                    g2, r = dt // 4, dt % 4
                    psh = pspool.tile([128, 512], FP, tag="mm")
                    for c in range(DC):
                        nc.tensor.matmul(
                            out=psh, lhsT=w1[c][g2][:, ds(128 * r, 128)], rhs=x_ds[c],
                            start=(c == 0), stop=(c == DC - 1),
                        )
                    ht = hpool.tile([128, 512], FR, tag="hT")
                    if b1_ap is not None:
                        nc.scalar.activation(out=ht, in_=psh, func=AF.Relu,
                                             bias=bias_col(b1_ap[ts(dt, 128)]))
                    else:
                        nc.scalar.activation(out=ht, in_=psh, func=AF.Relu)
                    hT.append(ht)
                z_sd = []
                for t in range(TT):
                    psf = pspool.tile([128, 512], FP, tag="mm")
                    for dt in range(FTL):
                        nc.tensor.matmul(
                            out=psf, lhsT=hT[dt][:, ts(t, 128)], rhs=w2[dt],
                            start=(dt == 0), stop=(dt == FTL - 1),
                        )
                    z_sd.append(residual(x_sd[t], psf, b2_ap))
                return z_sd

            # ================= encoder =================
            x_sd = []
            for t in range(TT):
                xt_ = apool.tile([128, 512], FR, tag="xn", bufs=8)
                nc.sync.dma_start(out=xt_, in_=x0e[ts(t, 128), :])
                x_sd.append(xt_)
            x_ds = transpose_sd(x_sd, "xT")
            if debug_stage == "x0":
                dump_and_stop(x_sd + x_ds)

            for i in range(LE):
                ab = be_attn[i] if flags.get("attn_bias") else None
                bo = be_attn[i, 3] if flags.get("ffn_bias") else None
                z = mha_any(x_sd, x_ds, x_ds, we_attn[i], ab, False, bcast, bo)
                lg = eln_g[i, 0] if flags.get("ln_affine") else None
                lb = eln_b[i, 0] if flags.get("ln_affine") else None
                x_sd, x_ds = ln_block(z, "xT", lg, lb)
                if debug_stage == f"enc{i}.ln1":
                    dump_and_stop(x_sd + x_ds)
                b1 = be_f1[i] if flags.get("ffn_bias") else None
                b2 = be_f2[i] if flags.get("ffn_bias") else None
                z = ffn(x_sd, x_ds, we_f1[i], we_f2[i], b1, b2)
                lg = eln_g[i, 1] if flags.get("ln_affine") else None
                lb = eln_b[i, 1] if flags.get("ln_affine") else None
                x_sd, x_ds = ln_block(z, "xT", lg, lb)
                if debug_stage == f"enc{i}":
                    dump_and_stop(x_sd + x_ds)

            # persist encoder output (transposed) for cross attention
            encT = []
            for c in range(DC):
                e = apool.tile([128, 512], FR, tag="encT", bufs=4)
                nc.any.tensor_copy(out=e, in_=x_ds[c])
                encT.append(e)

            # ================= decoder =================
            y_sd = []
            for t in range(TT):
                yt_ = apool.tile([128, 512], FR, tag="xn", bufs=8)
                nc.sync.dma_start(out=yt_, in_=x0d[ts(t, 128), :])
                y_sd.append(yt_)
            y_ds = transpose_sd(y_sd, "xT")

            for i in range(LD):
                ab = bd_sa[i] if flags.get("attn_bias") else None
                bo = bd_sa[i, 3] if flags.get("ffn_bias") else None
                z = mha_any(y_sd, y_ds, y_ds, wd_sa[i], ab, True, None, bo)
                lg = dln_g[i, 0] if flags.get("ln_affine") else None
                lb = dln_b[i, 0] if flags.get("ln_affine") else None
                y_sd, y_ds = ln_block(z, "xT", lg, lb)

                ab = bd_ca[i] if flags.get("attn_bias") else None
                bo = bd_ca[i, 3] if flags.get("ffn_bias") else None
                z = mha_any(y_sd, y_ds, encT, wd_ca[i], ab, False, bcast, bo)
                lg = dln_g[i, 1] if flags.get("ln_affine") else None
                lb = dln_b[i, 1] if flags.get("ln_affine") else None
                y_sd, y_ds = ln_block(z, "xT", lg, lb)

                b1 = bd_f1[i] if flags.get("ffn_bias") else None
                b2 = bd_f2[i] if flags.get("ffn_bias") else None
                z = ffn(y_sd, y_ds, wd_f1[i], wd_f2[i], b1, b2)
                lg = dln_g[i, 2] if flags.get("ln_affine") else None
                lb = dln_b[i, 2] if flags.get("ln_affine") else None
                y_sd, y_ds = ln_block(z, "xT", lg, lb)
                if debug_stage == f"dec{i}":
                    dump_and_stop(y_sd + y_ds)

            # ================= output projection =================
            nvc = (VH + 511) // 512
            for vc in range(nvc):
                w_ = min(512, VH - 512 * vc)
                wo_c = []
                for c in range(DC):
                    wt_ = wpool.tile([128, 512], FR, tag="w")
                    nc.sync.dma_start(out=wt_[:, :w_],
                                      in_=wout[ts(c, 128), ds(512 * vc, w_)])
                    wo_c.append(wt_)
                for t in range(TT):
                    pso = pspool.tile([128, 512], FP, tag="mm")
                    for c in range(DC):
                        nc.tensor.matmul(
                            out=pso[:, :w_], lhsT=y_ds[c][:, ts(t, 128)],
                            rhs=wo_c[c][:, :w_],
                            start=(c == 0), stop=(c == DC - 1),
                        )
                    sb = apool.tile([128, 512], FP, tag="ob", bufs=4)
                    if flags.get("out_bias"):
                        ob = bcast_tile(bout[ds(512 * vc, w_)], w_)
                        nc.vector.tensor_add(sb[:, :w_], pso[:, :w_], ob)
                    elif (vc * TT + t) % 2 == 0:
                        # alternate PSUM drains across ACT/DVE so the two
                        # mm slots recycle twice as fast in the logits tail
                        nc.scalar.copy(out=sb[:, :w_], in_=pso[:, :w_])
                    else:
                        nc.vector.tensor_copy(out=sb[:, :w_], in_=pso[:, :w_])
                    nc.sync.dma_start(out=logits[ts(t, 128), ds(512 * vc, w_)],
                                      in_=sb[:, :w_])
      except _StopTrace:
          pass

    nc.finalize()
    return nc


def _build_fast(debug_stage=None):
    """v2 builder: no masks/biases/affine-LN.  Single transposed activation
    layout xT:[D,S] (4 chunks of [128,512]).  No PE transposes anywhere:
      - LN stats via ones-column matmuls into one PSUM accumulation group
        ([2,512]: row0 = -mean, row1 = E[z^2]); scale/shift row broadcast via
        gpsimd; apply = 2 DVE passes per chunk.
      - attention out-proj and FFN w2 produce transposed outputs directly
        (lhsT = weight chunk, rhs = ctx/h in dim-partition layout), K=128.
      - decoder-causal scores packed two heads per PE pass via row-group
        tile_position (lhsT base partitions 0/64).
      - logits tail reuses the y stationary across 8 vocab chunks resident
        in all 8 PSUM banks.
    """
    nc = bacc.Bacc(None)

    x0eT = nc.declare_dram_parameter("x0eT", [D, S], FR, isOutput=False)
    x0dT = nc.declare_dram_parameter("x0dT", [D, S], FR, isOutput=False)
    we_attn = nc.declare_dram_parameter("we_attn", [LE, 4, D, D], FR, isOutput=False)
    we_f1 = nc.declare_dram_parameter("we_f1", [LE, D, DFF], FR, isOutput=False)
    we_f2 = nc.declare_dram_parameter("we_f2", [LE, DFF, D], FR, isOutput=False)
    wd_sa = nc.declare_dram_parameter("wd_sa", [LD, 4, D, D], FR, isOutput=False)
    wd_ca = nc.declare_dram_parameter("wd_ca", [LD, 4, D, D], FR, isOutput=False)
    wd_f1 = nc.declare_dram_parameter("wd_f1", [LD, D, DFF], FR, isOutput=False)
    wd_f2 = nc.declare_dram_parameter("wd_f2", [LD, DFF, D], FR, isOutput=False)
    wout = nc.declare_dram_parameter("wout", [D, VH], FR, isOutput=False)
    ctri = nc.declare_dram_parameter("ctri", [128, 128], FR, isOutput=False)
    cstat = nc.declare_dram_parameter("cstat", [128, 256], FR, isOutput=False)
    logits = nc.declare_dram_parameter("logits", [S, VH], FR, isOutput=True)
    dbg = None
    if debug_stage is not None:
        dbg = nc.declare_dram_parameter("dbg", [8, 128, 512], FP, isOutput=True)

    from contextlib import ExitStack

    with tile.TileContext(nc) as tc, ExitStack() as stk:
      try:
        wpool = stk.enter_context(tc.tile_pool(name="w", bufs=72))
        apool = stk.enter_context(tc.tile_pool(name="acts", bufs=2))
        hpool = stk.enter_context(tc.tile_pool(name="h", bufs=16))
        cpool = stk.enter_context(tc.tile_pool(name="consts", bufs=1))
        spool = stk.enter_context(tc.tile_pool(name="small", bufs=12))
        pspool = stk.enter_context(tc.tile_pool(name="ps", bufs=2, space="PSUM"))
        scpool = stk.enter_context(tc.tile_pool(name="sc", bufs=2, space="PSUM"))
        pcpool = stk.enter_context(tc.tile_pool(name="pc", bufs=2, space="PSUM"))
        stpool = stk.enter_context(tc.tile_pool(name="st", bufs=2, space="PSUM"))
        if True:
            # ---- constants --------------------------------------------------
            tri = cpool.tile([128, 128], FR)
            nc.sync.dma_start(out=tri, in_=ctri[:, :])
            statw = cpool.tile([128, 256], FR)
            nc.sync.dma_start(out=statw, in_=cstat[:, :])
            eps = cpool.tile([128, 1], FP)
            nc.vector.memset(eps, 1e-5)

            junk = cpool.tile([128, 1], FP, tag="junk")
            nc.vector.memset(junk, 1.0)

            def act_warm(func):
                # dummy activation to pull the ACT function table in while
                # the PE is busy, so the real op doesn't pay the ~1.3us
                # table reload on the LN critical path
                j = spool.tile([128, 1], FP, tag="jk", bufs=2)
                nc.scalar.activation(out=j, in_=junk, func=func)

            def wtile(dram_ap, tag="w"):
                t_ = wpool.tile([128, 512], FR, tag=tag)
                nc.sync.dma_start(out=t_, in_=dram_ap)
                return t_

            def dump_and_stop(tiles):
                for j, t_ in enumerate(tiles[:8]):
                    stt = apool.tile([128, 512], FP, tag="ob", bufs=4)
                    pp, ff = t_.shape[-2], t_.shape[-1]
                    nc.any.tensor_copy(out=stt[:pp, :ff], in_=t_)
                    nc.sync.dma_start(out=dbg[j, :pp, :ff], in_=stt[:pp, :ff])
                raise _StopTrace

            # ---- layernorm over the partition (D) axis ----------------------
            # z chunks are [128,512] bf16 (dims x tokens).  Stats:
            #   pstat[0,:] = sum_c (-1/512) * z_c   (= -mean)
            #   pstat[1,:] = sum_c (1/512) * z_c^2  (= E[z^2])
            # one PSUM accumulation group; two stationaries (cstat cols).
            def ln_T(z_ds, out_tag, out_bufs=8):
                # stats matmuls use a [128,128] constant stationary (+-1/512),
                # so every output partition carries the same row: the stats
                # arrive pre-broadcast and no gpsimd partition_broadcast is
                # needed anywhere in the chain.
                act_warm(AF.Abs_reciprocal_sqrt)
                pstatA = stpool.tile([128, 512], FP, tag="pstat")  # -mean
                pstatB = stpool.tile([128, 512], FP, tag="pstat")  # E[z^2]
                zsq = []
                for c in range(DC):
                    zq = apool.tile([128, 512], FR, tag="zsq", bufs=4)
                    nc.vector.tensor_mul(zq, z_ds[c], z_ds[c])
                    zsq.append(zq)
                for c in range(DC):
                    nc.tensor.matmul(
                        out=pstatA, lhsT=statw[:, 0:128], rhs=z_ds[c],
                        start=(c == 0), stop=(c == DC - 1),
                    )
                for c in range(DC):
                    nc.tensor.matmul(
                        out=pstatB, lhsT=statw[:, 128:256], rhs=zsq[c],
                        start=(c == 0), stop=(c == DC - 1),
                    )
                # centered z (zc = z - mu) runs off pstatA while the PE does
                # the sq stats matmuls; only the final scale multiply waits on
                # the rsqrt.
                zc_ds = []
                for c in range(DC):
                    zc = apool.tile([128, 512], FR, tag="zc", bufs=4)
                    nc.vector.tensor_add(zc, z_ds[c], pstatA)
                    zc_ds.append(zc)
                stage = apool.tile([128, 512], FP, tag="stg", bufs=2)
                nc.vector.tensor_copy(out=stage, in_=pstatA)
                musq = apool.tile([128, 512], FP, tag="musq", bufs=2)
                nc.vector.tensor_mul(musq, stage, stage)
                var = apool.tile([128, 512], FP, tag="var", bufs=2)
                nc.vector.tensor_sub(var, pstatB, musq)
                bcA = apool.tile([128, 512], FR, tag="lnbcA", bufs=2)
                nc.scalar.activation(out=bcA, in_=var,
                                     func=AF.Abs_reciprocal_sqrt, bias=eps)
                xn_ds = []
                for c in range(DC):
                    xn = apool.tile([128, 512], FR, tag=out_tag, bufs=out_bufs)
                    nc.vector.tensor_mul(xn, zc_ds[c], bcA)
                    xn_ds.append(xn)
                return xn_ds

            # ---- multi-head attention (transposed everything) ---------------
            # persistent V ring: per head 64 V-dim cols + 64 static ones cols
            # ([128, 8*128]); the ones make the ctx matmul emit the softmax
            # denominator pre-broadcast on psum partitions 64-127.
            NVR = 6
            vvo_ring = [apool.tile([128, 1024], FR, tag="vvo", bufs=NVR,
                                   name=f"vvo{i}") for i in range(NVR)]
            for i in range(NVR):
                nc.vector.memset(
                    _view3(vvo_ring[i][:, :], 8, 128, 64, inner_off=64), 1.0)
            vv_ctr = [0]
            mha_ctr = [0]

            def mha_T(xqT, xkvT, w_ap, causal):
                """Returns 4 z chunks [128,512] bf16 in [D,S] layout
                (attention output + residual xqT)."""
                midx = mha_ctr[0]; mha_ctr[0] += 1
                act_warm(AF.Exp)
                wq = [wtile(w_ap[0, ts(c, 128), :]) for c in range(DC)]
                wk = [wtile(w_ap[1, ts(c, 128), :]) for c in range(DC)]
                wv = [wtile(w_ap[2, ts(c, 128), :]) for c in range(DC)]
                wo = [wtile(w_ap[3, ts(p, 128), :]) for p in range(G)]

                QT, KT = [], []
                for g in range(G):
                    psq = pspool.tile([128, 512], FP, tag="mm")
                    for c in range(DC):
                        nc.tensor.matmul(
                            out=psq, lhsT=wq[c][:, ts(g, 128)], rhs=xqT[c],
                            start=(c == 0), stop=(c == DC - 1),
                        )
                    qt = apool.tile([128, 512], FR, tag="qt", bufs=5)
                    nc.vector.tensor_copy(out=qt, in_=psq)
                    QT.append(qt)
                    psk = pspool.tile([128, 512], FP, tag="mm")
                    for c in range(DC):
                        nc.tensor.matmul(
                            out=psk, lhsT=wk[c][:, ts(g, 128)], rhs=xkvT[c],
                            start=(c == 0), stop=(c == DC - 1),
                        )
                    kt = apool.tile([128, 512], FR, tag="kt", bufs=5)
                    nc.vector.tensor_copy(out=kt, in_=psk)
                    KT.append(kt)

                # V ring tiles: write the 8x64 V views; ones cols persist
                VO = []
                for t in range(TT):
                    psv = pspool.tile([128, 512], FP, tag="mm")
                    for c in range(DC):
                        nc.tensor.matmul(
                            out=psv, lhsT=xkvT[c][:, ts(t, 128)], rhs=wv[c],
                            start=(c == 0), stop=(c == DC - 1),
                        )
                    vvo = vvo_ring[vv_ctr[0] % NVR]
                    vv_ctr[0] += 1
                    nc.vector.tensor_copy(
                        out=_view3(vvo[:, :], 8, 128, 64),
                        in_=_view3(psv[:, :], 8, 64, 64),
                    )
                    VO.append(vvo)
                if debug_stage == f"mha{midx}.qkv":
                    dump_and_stop(QT + KT)
                if debug_stage == f"mha{midx}.v":
                    dump_and_stop(VO)

                ctxP = [apool.tile([128, 512], FR, tag="ctxP", bufs=6,
                                   name=f"ctxP{p}") for p in range(G)]
                for g in range(G):
                    # scores for the two heads of this pair run concurrently
                    # on PE row-groups 0-1 / 2-3 (lhsT base partition 0 / 64).
                    ES = [[], []]
                    for kt in range(TT):
                        q0 = 128 * kt if causal else 0
                        W = 512 - q0
                        scps = []
                        for lh in range(2):
                            qsl = slice(64 * lh, 64 * lh + 64)
                            scp = scpool.tile([128, 512], FP, tag="scp")
                            nc.tensor.matmul(
                                out=scp[:, ds(q0, W)],
                                lhsT=KT[g][qsl, ts(kt, 128)],
                                rhs=QT[g][qsl, ds(q0, W)],
                                start=True, stop=True,
                            )
                            scps.append(scp)
                        for lh in range(2):
                            scp = scps[lh]
                            if causal:
                                nc.vector.tensor_add(
                                    scp[:, ds(q0, 128)], scp[:, ds(q0, 128)], tri)
                            es = apool.tile([128, 512], FR, tag="exp", bufs=10)
                            nc.scalar.activation(out=es[:, ds(q0, W)],
                                                 in_=scp[:, ds(q0, W)], func=AF.Exp)
                            ES[lh].append(es)
                    for lh in range(2):
                        h = 2 * g + lh
                        psc = pcpool.tile([128, 512], FP, tag="psc")
                        for kt in range(TT):
                            q0 = 128 * kt if causal else 0
                            W = 512 - q0
                            nc.tensor.matmul(
                                out=psc[:, ds(q0, W)],
                                lhsT=VO[kt][:, ds(128 * h, 128)],
                                rhs=ES[lh][kt][:, ds(q0, W)],
                                start=(kt == 0), stop=(kt == TT - 1),
                            )
                        den = spool.tile([64, 512], FP, tag="den", bufs=4)
                        nc.vector.tensor_copy(out=den, in_=psc[64:128, :])
                        rec = spool.tile([64, 512], FP, tag="rec", bufs=4)
                        nc.vector.reciprocal_approx_fast(out=rec, in_=den)
                        nc.vector.tensor_mul(
                            ctxP[g][64 * lh: 64 * lh + 64, :],
                            psc[:64, :], rec)

                if debug_stage == f"mha{midx}.ctx":
                    dump_and_stop(ctxP)
                z_ds = []
                for c in range(DC):
                    pso = pspool.tile([128, 512], FP, tag="mm")
                    for p in range(G):
                        nc.tensor.matmul(
                            out=pso, lhsT=wo[p][:, ts(c, 128)], rhs=ctxP[p],
                            start=(p == 0), stop=(p == G - 1),
                        )
                    z = apool.tile([128, 512], FR, tag="z", bufs=4)
                    nc.vector.tensor_add(z, xqT[c], pso)
                    z_ds.append(z)
                if debug_stage == f"mha{midx}.out":
                    dump_and_stop(z_ds)
                return z_ds

            # ---- FFN (transposed output) ------------------------------------
            def ffn_T(xT, w1_ap, w2_ap):
                act_warm(AF.Relu)
                w1 = [[wtile(w1_ap[ts(c, 128), ts(g2, 512)]) for g2 in range(4)]
                      for c in range(DC)]
                w2 = [wtile(w2_ap[ts(dt, 128), :]) for dt in range(FTL)]
                hT = []
                for dt in range(FTL):
                    g2, r = dt // 4, dt % 4
                    psh = pspool.tile([128, 512], FP, tag="mm")
                    for c in range(DC):
                        nc.tensor.matmul(
                            out=psh, lhsT=w1[c][g2][:, ds(128 * r, 128)], rhs=xT[c],
                            start=(c == 0), stop=(c == DC - 1),
                        )
                    ht = hpool.tile([128, 512], FR, tag="hT")
                    nc.scalar.activation(out=ht, in_=psh, func=AF.Relu)
                    hT.append(ht)
                z_ds = []
                for c in range(DC):
                    psf = pspool.tile([128, 512], FP, tag="mm")
                    for dt in range(FTL):
                        nc.tensor.matmul(
                            out=psf, lhsT=w2[dt][:, ts(c, 128)], rhs=hT[dt],
                            start=(dt == 0), stop=(dt == FTL - 1),
                        )
                    z = apool.tile([128, 512], FR, tag="z", bufs=4)
                    nc.vector.tensor_add(z, xT[c], psf)
                    z_ds.append(z)
                return z_ds

            # ================= encoder =================
            x_ds = []
            for c in range(DC):
                xt_ = apool.tile([128, 512], FR, tag="xn", bufs=8)
                nc.sync.dma_start(out=xt_, in_=x0eT[ts(c, 128), :])
                x_ds.append(xt_)
            if debug_stage == "x0":
                dump_and_stop(x_ds)

            for i in range(LE):
                z = mha_T(x_ds, x_ds, we_attn[i], False)
                x_ds = ln_T(z, "xn")
                if debug_stage == f"enc{i}.ln1":
                    dump_and_stop(x_ds)
                z = ffn_T(x_ds, we_f1[i], we_f2[i])
                out_tag = "encT" if i == LE - 1 else "xn"
                x_ds = ln_T(z, out_tag, out_bufs=4 if i == LE - 1 else 8)
                if debug_stage == f"enc{i}":
                    dump_and_stop(x_ds)
            encT = x_ds

            # ================= decoder =================
            y_ds = []
            for c in range(DC):
                yt_ = apool.tile([128, 512], FR, tag="xn", bufs=8)
                nc.sync.dma_start(out=yt_, in_=x0dT[ts(c, 128), :])
                y_ds.append(yt_)

            for i in range(LD):
                z = mha_T(y_ds, y_ds, wd_sa[i], True)
                y_ds = ln_T(z, "xn")
                z = mha_T(y_ds, encT, wd_ca[i], False)
                y_ds = ln_T(z, "xn")
                z = ffn_T(y_ds, wd_f1[i], wd_f2[i])
                out_tag = "y" if i == LD - 1 else "xn"
                y_ds = ln_T(z, out_tag, out_bufs=4 if i == LD - 1 else 8)
                if debug_stage == f"dec{i}":
                    dump_and_stop(y_ds)

            # ================= output projection =================
            # 32 vocab chunks of <=512, in groups of 8 (one PSUM bank each);
            # stationary y[c][:, t-block] is reused across the 8 chunks.
            nvc = (VH + 511) // 512
            psum_of = [pspool, pspool, scpool, scpool,
                       pcpool, pcpool, stpool, stpool]
            ptag = ["mm", "mm", "scp", "scp", "psc", "psc", "pstat", "pstat"]
            for vg in range(0, nvc, 8):
                grp = list(range(vg, min(vg + 8, nvc)))
                wt_g = {}
                for c in range(DC):
                    for j in grp:
                        w_ = min(512, VH - 512 * j)
                        wt_ = wpool.tile([128, 512], FR, tag="w")
                        nc.sync.dma_start(out=wt_[:, :w_],
                                          in_=wout[ts(c, 128), ds(512 * j, w_)])
                        wt_g[(c, j)] = wt_
                for t in range(TT):
                    pss = [psum_of[k].tile([128, 512], FP, tag=ptag[k],
                                           name=f"pl{k}") for k in range(len(grp))]
                    for c in range(DC):
                        for k, j in enumerate(grp):
                            w_ = min(512, VH - 512 * j)
                            nc.tensor.matmul(
                                out=pss[k][:, :w_],
                                lhsT=y_ds[c][:, ts(t, 128)],
                                rhs=wt_g[(c, j)][:, :w_],
                                start=(c == 0), stop=(c == DC - 1),
                            )
                    for k, j in enumerate(grp):
                        w_ = min(512, VH - 512 * j)
                        sb = apool.tile([128, 512], FR, tag="ob", bufs=8)
                        if k % 2 == 0:
                            nc.scalar.copy(out=sb[:, :w_], in_=pss[k][:, :w_])
                        else:
                            nc.vector.tensor_copy(out=sb[:, :w_], in_=pss[k][:, :w_])
                        nc.sync.dma_start(out=logits[ts(t, 128), ds(512 * j, w_)],
                                          in_=sb[:, :w_])
      except _StopTrace:
          pass

    nc.finalize()
    return nc


def _host_prep(inputs):
    """Host-side preparation: embeddings, weight folding, masks, flags."""
    src = np.asarray(inputs["src"])
    tgt = np.asarray(inputs["tgt"])
    f32 = lambda k: np.ascontiguousarray(np.asarray(inputs[k], dtype=np.float32))

    enc_emb, dec_emb, pe = f32("enc_emb"), f32("dec_emb"), f32("pe")
    sqd = np.float32(math.sqrt(D))
    x0e = enc_emb[src] * sqd + pe[None, :S]          # [B, S, D]
    x0d = dec_emb[tgt] * sqd + pe[None, :S]

    we_attn = f32("enc_attn_w").copy()
    wd_sa = f32("dec_sa_w").copy()
    wd_ca = f32("dec_ca_w").copy()
    scale = np.float32(1.0 / math.sqrt(DK))
    we_attn[:, 0] *= scale
    wd_sa[:, 0] *= scale
    wd_ca[:, 0] *= scale
    be_attn = f32("enc_attn_b").copy()
    bd_sa = f32("dec_sa_b").copy()
    bd_ca = f32("dec_ca_b").copy()
    be_attn[:, 0] *= scale
    bd_sa[:, 0] *= scale
    bd_ca[:, 0] *= scale

    flags = {
        "attn_bias": bool(np.any(be_attn[:, (0, 2)]) or np.any(bd_sa[:, (0, 2)])
                          or np.any(bd_ca[:, (0, 2)])),
        "ffn_bias": bool(np.any(f32("enc_ffn_b1")) or np.any(f32("enc_ffn_b2"))
                         or np.any(f32("dec_ffn_b1")) or np.any(f32("dec_ffn_b2"))
                         or np.any(be_attn[:, 3]) or np.any(bd_sa[:, 3])
                         or np.any(bd_ca[:, 3])),
        "ln_affine": bool(np.any(f32("enc_ln_g") != 1.0) or np.any(f32("enc_ln_b"))
                          or np.any(f32("dec_ln_g") != 1.0) or np.any(f32("dec_ln_b"))),
        "out_bias": bool(np.any(f32("out_b"))),
        "src_mask": bool((src == 0).any()),
        "tgt_mask": bool((tgt == 0).any()),
    }

    bf = lambda a: np.ascontiguousarray(np.asarray(a, np.float32).astype(BFNP))
    cident = np.eye(128, dtype=np.float32)
    ctri = np.tril(np.full((128, 128), NEG, dtype=np.float32), k=-1)

    common = {
        "we_attn": bf(we_attn),
        "we_f1": bf(f32("enc_ffn_w1")), "we_f2": bf(f32("enc_ffn_w2")),
        "wd_sa": bf(wd_sa), "wd_ca": bf(wd_ca),
        "wd_f1": bf(f32("dec_ffn_w1")), "wd_f2": bf(f32("dec_ffn_w2")),
        "cident": bf(cident), "ctri": bf(ctri),
        "cones": np.ones((1, 64), np.float32),
    }
    if flags["attn_bias"]:
        common.update(be_attn=np.ascontiguousarray(be_attn),
                      bd_sa=np.ascontiguousarray(bd_sa),
                      bd_ca=np.ascontiguousarray(bd_ca))
    if flags["ffn_bias"]:
        common.update(be_f1=f32("enc_ffn_b1"), be_f2=f32("enc_ffn_b2"),
                      bd_f1=f32("dec_ffn_b1"), bd_f2=f32("dec_ffn_b2"))
    if flags["ln_affine"]:
        common.update(eln_g=f32("enc_ln_g"), eln_b=f32("enc_ln_b"),
                      dln_g=f32("dec_ln_g"), dln_b=f32("dec_ln_b"))

    out_w = f32("out_w")
    out_b = f32("out_b")

    in_maps = []
    for core in range(NCORES):
        b, half = core // 2, core % 2
        m = dict(common)
        m["x0e"] = bf(x0e[b])
        m["x0d"] = bf(x0d[b])
        m["wout"] = bf(out_w[:, half * VH : (half + 1) * VH])
        if flags["out_bias"]:
            m["bout"] = np.ascontiguousarray(out_b[half * VH : (half + 1) * VH])
        if flags["src_mask"]:
            m["km_src"] = np.where(src[b] != 0, 0.0, NEG).astype(np.float32)
        if flags["tgt_mask"]:
            rm = (tgt[b] != 0).astype(np.float32)
            m["rm_tgt"] = rm
            cm = np.where(np.tril(np.ones((S, S), bool)), 0.0, NEG).astype(np.float32)
            cm = cm * rm[:, None]          # padded query rows -> all-zero scores
            m["cm_tgt"] = np.ascontiguousarray(cm)
        in_maps.append(m)

    return flags, in_maps


def _any_special(inputs):
    """True if any mask/bias/affine feature is active (v1 fallback needed)."""
    f32 = lambda k: np.asarray(inputs[k], dtype=np.float32)
    src = np.asarray(inputs["src"])
    tgt = np.asarray(inputs["tgt"])
    return bool(
        np.any(f32("enc_attn_b")) or np.any(f32("dec_sa_b"))
        or np.any(f32("dec_ca_b"))
        or np.any(f32("enc_ffn_b1")) or np.any(f32("enc_ffn_b2"))
        or np.any(f32("dec_ffn_b1")) or np.any(f32("dec_ffn_b2"))
        or np.any(f32("enc_ln_g") != 1.0) or np.any(f32("enc_ln_b"))
        or np.any(f32("dec_ln_g") != 1.0) or np.any(f32("dec_ln_b"))
        or np.any(f32("out_b"))
        or (src == 0).any() or (tgt == 0).any()
    )


def _host_prep_fast(inputs):
    src = np.asarray(inputs["src"])
    tgt = np.asarray(inputs["tgt"])
    f32 = lambda k: np.ascontiguousarray(np.asarray(inputs[k], dtype=np.float32))

    enc_emb, dec_emb, pe = f32("enc_emb"), f32("dec_emb"), f32("pe")
    sqd = np.float32(math.sqrt(D))
    x0e = enc_emb[src] * sqd + pe[None, :S]          # [B, S, D]
    x0d = dec_emb[tgt] * sqd + pe[None, :S]

    we_attn = f32("enc_attn_w").copy()
    wd_sa = f32("dec_sa_w").copy()
    wd_ca = f32("dec_ca_w").copy()
    scale = np.float32(1.0 / math.sqrt(DK))
    we_attn[:, 0] *= scale
    wd_sa[:, 0] *= scale
    wd_ca[:, 0] *= scale

    bf = lambda a: np.ascontiguousarray(np.asarray(a, np.float32).astype(BFNP))
    ctri = np.tril(np.full((128, 128), NEG, dtype=np.float32), k=-1)
    cstat = np.zeros((128, 256), np.float32)
    cstat[:, 0:128] = -1.0 / 512
    cstat[:, 128:256] = 1.0 / 512

    common = {
        "we_attn": bf(we_attn),
        "we_f1": bf(f32("enc_ffn_w1")), "we_f2": bf(f32("enc_ffn_w2")),
        "wd_sa": bf(wd_sa), "wd_ca": bf(wd_ca),
        "wd_f1": bf(f32("dec_ffn_w1")), "wd_f2": bf(f32("dec_ffn_w2")),
        "ctri": bf(ctri), "cstat": bf(cstat),
    }
    out_w = f32("out_w")
    in_maps = []
    for core in range(NCORES):
        b, half = core // 2, core % 2
        m = dict(common)
        m["x0eT"] = bf(np.ascontiguousarray(x0e[b].T))
        m["x0dT"] = bf(np.ascontiguousarray(x0d[b].T))
        m["wout"] = bf(out_w[:, half * VH: (half + 1) * VH])
        in_maps.append(m)
    return in_maps


def _run(inputs, trace=False, debug_stage=None, **kwargs):
    if not _any_special(inputs):
        in_maps = _host_prep_fast(inputs)
        nc = _build_fast(debug_stage)
        res = run_bass_kernel_spmd(nc, in_maps, list(range(NCORES)),
                                   trace=trace, **kwargs)
        out = np.zeros((4, S, V), dtype=np.float32)
        for core in range(NCORES):
            b, half = core // 2, core % 2
            out[b, :, half * VH: (half + 1) * VH] = np.asarray(
                res.results[core]["logits"], np.float32)
        return out, res
    flags, in_maps = _host_prep(inputs)
    if flags["ffn_bias"] and not flags["attn_bias"]:
        # mha's bo path reads attn-bias tensors; force-declare them
        flags["attn_bias"] = True
        be = np.asarray(inputs["enc_attn_b"], np.float32).copy()
        be[:, 0] *= np.float32(1.0 / math.sqrt(DK))
        bs = np.asarray(inputs["dec_sa_b"], np.float32).copy()
        bs[:, 0] *= np.float32(1.0 / math.sqrt(DK))
        bc = np.asarray(inputs["dec_ca_b"], np.float32).copy()
        bc[:, 0] *= np.float32(1.0 / math.sqrt(DK))
        for m in in_maps:
            m.update(be_attn=np.ascontiguousarray(be),
                     bd_sa=np.ascontiguousarray(bs),
                     bd_ca=np.ascontiguousarray(bc))
    nc = _build_program(flags)
    res = run_bass_kernel_spmd(nc, in_maps, list(range(NCORES)), trace=trace, **kwargs)
    out = np.zeros((4, S, V), dtype=np.float32)
    for core in range(NCORES):
        b, half = core // 2, core % 2
        out[b, :, half * VH : (half + 1) * VH] = res.results[core]["logits"]
    return out, res


def kernel(**inputs):
    out, _ = _run(inputs, trace=False)
    return out



# revision 22
# speedup vs baseline: 1.0198x; 1.0198x over previous
"""Trainium2 Bass kernel for nn_CustomTransformer_64570538328578.

Encoder-decoder transformer: V=32000, D=512, H=8, L=6+6, DFF=2048, B=4, S=512.

Sharding: 8 cores = 4 batch pairs x 2 vocab halves.  Core c handles batch
element c//2 (full encoder+decoder stack, duplicated within the pair) and
computes logits for vocab half c%2 of the output projection.  No on-device
collectives needed.

All matmul data is bf16 (PSUM accumulation fp32).  Attention is computed in
transposed-score layout: ST[k,q] = (K^T)^T-by-Q products per k-tile, exp on
ACT, and the softmax denominator comes for free from a ones-column appended
to the V stationary (65-row ctx matmul).  The per-query reciprocal is
broadcast across partitions with a rank-1 fp32r matmul, so no per-head
transposes of the attention matrix are needed at all.

Layouts on device (per core):
  - canonical activations x: [S, D] as 4 tiles [128, 512] (token-partition)
  - transposed activations xT: [D, S] as 4 tiles [128, 512] (dim-partition)
  - per-head QT/KT: [DK, S] packed 2 heads/tile -> 4 tiles [128, 512]
  - V+ones: [S, 8*65] 4 tiles (per-head 64 dims + ones col side by side)
  - scoresT per (head, k-tile): PSUM [128, <=512]; causal diag mask added
    via an ident x tril(-1e9) matmul into the first 128 columns.
"""

import math
import sys

import ml_dtypes
import numpy as np

if "/opt/trn_rl_repo" not in sys.path:
    sys.path.insert(0, "/opt/trn_rl_repo")

import concourse.bass as bass
import concourse.tile as tile
from concourse import bacc
from concourse import mybir
from concourse.bass import ds, ts
from concourse.bass_utils import run_bass_kernel_spmd

FP = mybir.dt.float32
F32R = mybir.dt.float32r
S = 512
D = 512
H = 8
DK = 64
DFF = 2048
LE = 6
LD = 6
V = 32000
NCORES = 8
VH = V // 2        # vocab half per core
TT = 4             # token tiles (S / 128)
DC = 4             # D chunks of 128
G = 4              # head-pair groups (2 heads of 64 dims -> 128 partitions)
FTL = 16           # dff tiles of 128
NEG = -1.0e9
AF = mybir.ActivationFunctionType
FR = mybir.dt.bfloat16   # matmul/activation storage dtype
BFNP = ml_dtypes.bfloat16


def _f32r(ap):
    return ap.bitcast(F32R)


def _view3(ap, groups, gstride, inner, inner_off=0):
    """[128, x] AP -> [128, groups, inner] view with group stride gstride."""
    a = ap
    return bass.AP(
        tensor=a.tensor,
        offset=a.offset + inner_off,
        ap=[a.ap[0], [gstride, groups], [1, inner]],
    )


class _StopTrace(Exception):
    pass


def _build_program(flags, debug_stage=None):
    """Build the single SPMD Bass program (same for all cores)."""
    nc = bacc.Bacc(None)

    # ---- DRAM parameters ----------------------------------------------------
    x0e = nc.declare_dram_parameter("x0e", [S, D], FR, isOutput=False)
    x0d = nc.declare_dram_parameter("x0d", [S, D], FR, isOutput=False)
    we_attn = nc.declare_dram_parameter("we_attn", [LE, 4, D, D], FR, isOutput=False)
    we_f1 = nc.declare_dram_parameter("we_f1", [LE, D, DFF], FR, isOutput=False)
    we_f2 = nc.declare_dram_parameter("we_f2", [LE, DFF, D], FR, isOutput=False)
    wd_sa = nc.declare_dram_parameter("wd_sa", [LD, 4, D, D], FR, isOutput=False)
    wd_ca = nc.declare_dram_parameter("wd_ca", [LD, 4, D, D], FR, isOutput=False)
    wd_f1 = nc.declare_dram_parameter("wd_f1", [LD, D, DFF], FR, isOutput=False)
    wd_f2 = nc.declare_dram_parameter("wd_f2", [LD, DFF, D], FR, isOutput=False)
    wout = nc.declare_dram_parameter("wout", [D, VH], FR, isOutput=False)
    cident = nc.declare_dram_parameter("cident", [128, 128], FR, isOutput=False)
    ctri = nc.declare_dram_parameter("ctri", [128, 128], FR, isOutput=False)
    cones = nc.declare_dram_parameter("cones", [1, 64], F32R, isOutput=False)
    logits = nc.declare_dram_parameter("logits", [S, VH], FP, isOutput=True)
    dbg = None
    if debug_stage is not None:
        dbg = nc.declare_dram_parameter("dbg", [8, 128, 512], FP, isOutput=True)

    # optional (general-path) params, declared only when actually used
    if flags.get("attn_bias"):
        be_attn = nc.declare_dram_parameter("be_attn", [LE, 4, D], FP, isOutput=False)
        bd_sa = nc.declare_dram_parameter("bd_sa", [LD, 4, D], FP, isOutput=False)
        bd_ca = nc.declare_dram_parameter("bd_ca", [LD, 4, D], FP, isOutput=False)
    if flags.get("ffn_bias"):
        be_f1 = nc.declare_dram_parameter("be_f1", [LE, DFF], FP, isOutput=False)
        be_f2 = nc.declare_dram_parameter("be_f2", [LE, D], FP, isOutput=False)
        bd_f1 = nc.declare_dram_parameter("bd_f1", [LD, DFF], FP, isOutput=False)
        bd_f2 = nc.declare_dram_parameter("bd_f2", [LD, D], FP, isOutput=False)
    if flags.get("ln_affine"):
        eln_g = nc.declare_dram_parameter("eln_g", [LE, 2, D], FP, isOutput=False)
        eln_b = nc.declare_dram_parameter("eln_b", [LE, 2, D], FP, isOutput=False)
        dln_g = nc.declare_dram_parameter("dln_g", [LD, 3, D], FP, isOutput=False)
        dln_b = nc.declare_dram_parameter("dln_b", [LD, 3, D], FP, isOutput=False)
    if flags.get("out_bias"):
        bout = nc.declare_dram_parameter("bout", [VH], FP, isOutput=False)
    if flags.get("src_mask"):
        km_src = nc.declare_dram_parameter("km_src", [S], FP, isOutput=False)
    if flags.get("tgt_mask"):
        rm_tgt = nc.declare_dram_parameter("rm_tgt", [S], FP, isOutput=False)
        cm_tgt = nc.declare_dram_parameter("cm_tgt", [S, S], FP, isOutput=False)

    from contextlib import ExitStack

    with tile.TileContext(nc) as tc, ExitStack() as stk:
      try:
        wpool = stk.enter_context(tc.tile_pool(name="w", bufs=64))
        apool = stk.enter_context(tc.tile_pool(name="acts", bufs=2))
        hpool = stk.enter_context(tc.tile_pool(name="h", bufs=16))
        cpool = stk.enter_context(tc.tile_pool(name="consts", bufs=1))
        spool = stk.enter_context(tc.tile_pool(name="small", bufs=12))
        pspool = stk.enter_context(tc.tile_pool(name="ps", bufs=2, space="PSUM"))
        pcpool = stk.enter_context(tc.tile_pool(name="pc", bufs=3, space="PSUM"))
        scpool = stk.enter_context(tc.tile_pool(name="sc", bufs=3, space="PSUM"))
        if True:

            # ---- constants --------------------------------------------------
            ident = cpool.tile([128, 128], FR)
            nc.sync.dma_start(out=ident, in_=cident[:, :])
            tri = cpool.tile([128, 128], FR)
            nc.sync.dma_start(out=tri, in_=ctri[:, :])
            eps = cpool.tile([128, 1], FP)
            nc.vector.memset(eps, 1e-5)
            ones64 = cpool.tile([1, 64], F32R)
            nc.sync.dma_start(out=ones64, in_=cones[:, :])

            bcast = None
            if flags.get("src_mask"):
                bcast = cpool.tile([128, S], FP)
                kma = km_src[:]
                nc.sync.dma_start(
                    out=bcast,
                    in_=bass.AP(
                        tensor=kma.tensor,
                        offset=kma.offset,
                        ap=[[0, 128]] + kma.ap,
                    ),
                )
            rmt = None
            if flags.get("tgt_mask"):
                rmt = cpool.tile([128, TT], FP)
                for t in range(TT):
                    nc.sync.dma_start(out=rmt[:, t : t + 1], in_=rm_tgt[ts(t, 128)])
                cmt = []
                for t in range(TT):
                    cm = cpool.tile([128, S], FP, tag="cmt")
                    nc.sync.dma_start(out=cm, in_=cm_tgt[ts(t, 128), :])
                    cmt.append(cm)

            junk = cpool.tile([128, 1], FP, tag="junk")
            nc.vector.memset(junk, 1.0)

            def act_warm(func):
                # dummy activation to pull the ACT function table in while
                # the PE is busy, so the real op doesn't pay the ~1.3us
                # table reload on the LN critical path
                j = spool.tile([128, 1], FP, tag="jk", bufs=2)
                nc.scalar.activation(out=j, in_=junk, func=func)

            def wtile(dram_ap, tag="w"):
                t_ = wpool.tile([128, 512], FR, tag=tag)
                rows = dram_ap.shape[-2]
                nc.sync.dma_start(out=t_[:rows, :], in_=dram_ap)
                return t_

            def bias_col(dram_ap):
                n = dram_ap.shape[-1]
                b = spool.tile([128, 1], FP, tag="bias")
                nc.sync.dma_start(out=b[:n, :], in_=dram_ap)
                return b[:n, :]

            def bcast_tile(dram_ap, n):
                """[n] dram vector -> [128, n] sbuf tile (partition broadcast)."""
                b = apool.tile([128, n], FP, tag="bc", bufs=2)
                nc.sync.dma_start(
                    out=b,
                    in_=bass.AP(
                        tensor=dram_ap.tensor,
                        offset=dram_ap.offset,
                        ap=[[0, 128]] + dram_ap.ap,
                    ),
                )
                return b

            def dump_and_stop(tiles):
                for j, t_ in enumerate(tiles[:8]):
                    stt = apool.tile([128, 512], FP, tag="ob", bufs=4)
                    pp, ff = t_.shape[-2], t_.shape[-1]
                    nc.any.tensor_copy(out=stt[:pp, :ff], in_=t_)
                    nc.sync.dma_start(out=dbg[j, :pp, :ff], in_=stt[:pp, :ff])
                raise _StopTrace

            # ---- transpose helper: [S,D] tiles -> [D,S] tiles ---------------
            def transpose_sd(sd_tiles, out_tag):
                ds_tiles = []
                for c in range(DC):
                    trp = scpool.tile([128, 512], FR, tag="scp")
                    for t in range(TT):
                        nc.tensor.transpose(
                            out=trp[:, ds(128 * t, 128)],
                            in_=sd_tiles[t][:, ds(128 * c, 128)],
                            identity=ident,
                        )
                    xt = apool.tile([128, 512], FR, tag=out_tag, bufs=8)
                    # ACT is idle during the LN phase; keep DVE free for stats
                    nc.scalar.copy(out=xt, in_=trp)
                    ds_tiles.append(xt)
                return ds_tiles

            # ---- layernorm (input z already includes the residual) ----------
            def ln_block(z_sd, xt_tag, g_ap=None, b_ap=None):
                new_sd = []
                for t in range(TT):
                    z = z_sd[t]
                    st6 = spool.tile([128, 6], FP, tag="st6")
                    nc.vector.bn_stats(out=st6, in_=z)
                    mv = spool.tile([128, 2], FP, tag="mv")
                    nc.vector.bn_aggr(out=mv, in_=st6)
                    sd_ = spool.tile([128, 1], FP, tag="sd")
                    nc.scalar.activation(out=sd_, in_=mv[:, 1:2], func=AF.Sqrt, bias=eps)
                    rr = spool.tile([128, 1], FP, tag="rr")
                    nc.vector.reciprocal(out=rr, in_=sd_)
                    xn = apool.tile([128, 512], FR, tag="xn", bufs=8)
                    nc.vector.tensor_scalar(
                        out=xn, in0=z, scalar1=mv[:, 0:1], scalar2=rr,
                        op0=mybir.AluOpType.subtract, op1=mybir.AluOpType.mult,
                    )
                    if g_ap is not None:
                        gt_ = bcast_tile(g_ap, 512)
                        nc.vector.tensor_mul(xn, xn, gt_)
                    if b_ap is not None:
                        bt_ = bcast_tile(b_ap, 512)
                        nc.vector.tensor_add(xn, xn, bt_)
                    new_sd.append(xn)
                return new_sd, transpose_sd(new_sd, xt_tag)

            def residual(x_t, ps, badd_ap=None):
                """z = x_t + ps (+ badd broadcast); returns SBUF bf16 tile."""
                z = apool.tile([128, 512], FR, tag="z", bufs=4)
                nc.vector.tensor_add(z, x_t, ps)
                if badd_ap is not None:
                    bt = bcast_tile(badd_ap, 512)
                    nc.vector.tensor_add(z, z, bt)
                return z

            # ---- multi-head attention, transposed-score layout --------------
            mha_ctr = [0]

            def mha(x_sd, xq_ds, xkv_ds, w_ap, causal, badd_ap=None):
                """Fast path (no padding masks, no attn biases).
                Returns 4 SBUF z tiles (attn output + residual)."""
                midx = mha_ctr[0]; mha_ctr[0] += 1
                act_warm(AF.Exp)
                wq = [wtile(w_ap[0, ts(c, 128), :]) for c in range(DC)]
                wk = [wtile(w_ap[1, ts(c, 128), :]) for c in range(DC)]
                wv = [wtile(w_ap[2, ts(c, 128), :]) for c in range(DC)]
                wo = [wtile(w_ap[3, ds(64 * h, 64), :]) for h in range(H)]

                QT, KT = [], []
                for g in range(G):
                    psq = pspool.tile([128, 512], FP, tag="mm")
                    for c in range(DC):
                        nc.tensor.matmul(
                            out=psq, lhsT=wq[c][:, ts(g, 128)], rhs=xq_ds[c],
                            start=(c == 0), stop=(c == DC - 1),
                        )
                    qt = apool.tile([128, 512], FR, tag="qt", bufs=5)
                    nc.vector.tensor_copy(out=qt, in_=psq)
                    QT.append(qt)
                    psk = pspool.tile([128, 512], FP, tag="mm")
                    for c in range(DC):
                        nc.tensor.matmul(
                            out=psk, lhsT=wk[c][:, ts(g, 128)], rhs=xkv_ds[c],
                            start=(c == 0), stop=(c == DC - 1),
                        )
                    kt = apool.tile([128, 512], FR, tag="kt", bufs=5)
                    nc.vector.tensor_copy(out=kt, in_=psk)
                    KT.append(kt)

                # V with a ones column appended per head: [128 tok, 8*65]
                VO = []
                for t in range(TT):
                    psv = pspool.tile([128, 512], FP, tag="mm")
                    for c in range(DC):
                        nc.tensor.matmul(
                            out=psv, lhsT=xkv_ds[c][:, ts(t, 128)], rhs=wv[c],
                            start=(c == 0), stop=(c == DC - 1),
                        )
                    vvo = apool.tile([128, 8 * 65], FR, tag="vvo", bufs=5)
                    va = vvo[:, :]
                    nc.vector.memset(
                        bass.AP(tensor=va.tensor, offset=va.offset + 64,
                                ap=[va.ap[0], [65, 8], [1, 1]]), 1.0)
                    nc.vector.tensor_copy(
                        out=_view3(va, 8, 65, 64),
                        in_=_view3(psv[:, :], 8, 64, 64),
                    )
                    VO.append(vvo)
                if debug_stage == f"mha{midx}.qkv":
                    dump_and_stop(QT + KT)
                if debug_stage == f"mha{midx}.v":
                    dump_and_stop(VO)

                ctxT = []
                for g in range(G):
                    pscs = []
                    for lh in range(2):
                        h = 2 * g + lh
                        qsl = slice(64 * lh, 64 * lh + 64)
                        ES = []
                        for kt in range(TT):
                            q0 = 128 * kt if causal else 0
                            W = 512 - q0
                            scp = scpool.tile([128, 512], FP, tag="scp")
                            nc.tensor.matmul(
                                out=scp[:, ds(q0, W)],
                                lhsT=KT[g][qsl, ts(kt, 128)],
                                rhs=QT[g][qsl, ds(q0, W)],
                                start=True, stop=True,
                            )
                            if causal:
                                # add tril(-1e9, -1) to the diagonal block on
                                # DVE (frees a PE matmul+ldweights per k-tile)
                                nc.vector.tensor_add(
                                    scp[:, ds(q0, 128)], scp[:, ds(q0, 128)], tri)
                            es = apool.tile([128, 512], FR, tag="exp", bufs=8)
                            nc.scalar.activation(out=es[:, ds(q0, W)],
                                                 in_=scp[:, ds(q0, W)], func=AF.Exp)
                            ES.append(es)
                        psc = pcpool.tile([128, 512], FP, tag="psc")
                        for kt in range(TT):
                            q0 = 128 * kt if causal else 0
                            W = 512 - q0
                            nc.tensor.matmul(
                                out=psc[:65, ds(q0, W)],
                                lhsT=VO[kt][:, ds(65 * h, 65)],
                                rhs=ES[kt][:, ds(q0, W)],
                                start=(kt == 0), stop=(kt == TT - 1),
                            )
                        # softmax denominator: reciprocal of the ones-row, then
                        # partition-broadcast on the (idle) gpsimd engine
                        rec = spool.tile([1, 512], FR, tag="rec", bufs=4)
                        with nc.allow_low_precision(reason="bf16 softmax recip"):
                            nc.vector.reciprocal(out=rec, in_=psc[64:65, :])
                        rbs = apool.tile([64, 512], FR, tag="rbs", bufs=4)
                        nc.gpsimd.partition_broadcast(rbs[:, :], rec[:, :])
                        ct = apool.tile([64, 512], FR, tag="ctxh", bufs=10)
                        nc.vector.tensor_mul(ct, psc[:64, :], rbs)
                        ctxT.append(ct)

                if debug_stage == f"mha{midx}.ctx":
                    dump_and_stop(ctxT)
                z_sd = []
                for t in range(TT):
                    pso = pspool.tile([128, 512], FP, tag="mm")
                    for h in range(H):
                        nc.tensor.matmul(
                            out=pso, lhsT=ctxT[h][:, ts(t, 128)],
                            rhs=wo[h][:64, :],
                            start=(h == 0), stop=(h == H - 1),
                        )
                    z_sd.append(residual(x_sd[t], pso, badd_ap))
                if debug_stage == f"mha{midx}.out":
                    dump_and_stop(z_sd)
                return z_sd

            # ---- masked multi-head attention (general path) -----------------
            def mha_masked(x_sd, xq_ds, xkv_ds, w_ap, b_ap, causal, kmask,
                           badd_ap=None):
                midx = mha_ctr[0]; mha_ctr[0] += 1
                act_warm(AF.Exp)
                wq = [wtile(w_ap[0, ts(c, 128), :]) for c in range(DC)]
                wk = [wtile(w_ap[1, ts(c, 128), :]) for c in range(DC)]
                wv = [wtile(w_ap[2, ts(c, 128), :]) for c in range(DC)]
                wo = [wtile(w_ap[3, ds(64 * h, 64), :]) for h in range(H)]

                QT, KT = [], []
                for g in range(G):
                    psq = pspool.tile([128, 512], FP, tag="mm")
                    for c in range(DC):
                        nc.tensor.matmul(
                            out=psq, lhsT=wq[c][:, ts(g, 128)], rhs=xq_ds[c],
                            start=(c == 0), stop=(c == DC - 1),
                        )
                    qt = apool.tile([128, 512], FR, tag="qt", bufs=5)
                    if b_ap is not None:
                        nc.scalar.activation(out=qt, in_=psq, func=AF.Identity,
                                             bias=bias_col(b_ap[0, ts(g, 128)]))
                    else:
                        nc.scalar.copy(out=qt, in_=psq)
                    QT.append(qt)
                for g in range(G):
                    psk = pspool.tile([128, 512], FP, tag="mm")
                    for c in range(DC):
                        nc.tensor.matmul(
                            out=psk, lhsT=wk[c][:, ts(g, 128)], rhs=xkv_ds[c],
                            start=(c == 0), stop=(c == DC - 1),
                        )
                    kt = apool.tile([128, 512], FR, tag="kt", bufs=5)
                    if b_ap is not None:
                        nc.scalar.activation(out=kt, in_=psk, func=AF.Identity,
                                             bias=bias_col(b_ap[1, ts(g, 128)]))
                    else:
                        nc.scalar.copy(out=kt, in_=psk)
                    KT.append(kt)
                VV = []
                for t in range(TT):
                    psv = pspool.tile([128, 512], FP, tag="mm")
                    for c in range(DC):
                        nc.tensor.matmul(
                            out=psv, lhsT=xkv_ds[c][:, ts(t, 128)], rhs=wv[c],
                            start=(c == 0), stop=(c == DC - 1),
                        )
                    vv = apool.tile([128, 512], FR, tag="vvo", bufs=5)
                    if b_ap is not None:
                        bt = bcast_tile(b_ap[2], 512)
                        nc.vector.tensor_add(vv, psv, bt)
                    else:
                        nc.scalar.copy(out=vv, in_=psv)
                    VV.append(vv)

                ctxT = []
                for g in range(G):
                    for lh in range(2):
                        h = 2 * g + lh
                        psc = pcpool.tile([128, 512], FP, tag="psc")
                        qsl = slice(64 * lh, 64 * lh + 64)
                        att = []
                        for t in range(TT):
                            nch = TT if not causal else (t + 1)
                            W = 128 * nch
                            scp = scpool.tile([128, 512], FP, tag="scp")
                            nc.tensor.matmul(
                                out=scp[:, :W],
                                lhsT=QT[g][qsl, ts(t, 128)],
                                rhs=KT[g][qsl, :W],
                                start=True, stop=not causal,
                            )
                            if causal:
                                nc.tensor.matmul(
                                    out=scp[:, ds(128 * t, 128)],
                                    lhsT=tri, rhs=ident,
                                    start=False, stop=True,
                                )
                            if kmask is not None:
                                nc.vector.tensor_add(scp[:, :W], scp[:, :W], kmask[:, :W])
                            if causal and flags.get("tgt_mask"):
                                nc.vector.tensor_scalar_mul(scp[:, :W], scp[:, :W],
                                                            rmt[:, t : t + 1])
                                nc.vector.tensor_add(scp[:, :W], scp[:, :W], cmt[t][:, :W])
                            et = apool.tile([128, 512], FR, tag="exp", bufs=8)
                            ssum = spool.tile([128, 1], FP, tag="ssum")
                            nc.scalar.activation(out=et[:, :W], in_=scp[:, :W],
                                                 func=AF.Exp, accum_out=ssum)
                            rr = spool.tile([128, 1], FP, tag="srr")
                            nc.vector.reciprocal(out=rr, in_=ssum)
                            nc.vector.tensor_scalar_mul(et[:, :W], et[:, :W], rr)
                            att.append(et)
                        for c in range(TT):
                            t0 = c if causal else 0
                            wq_ = 512 - 128 * t0
                            trp = scpool.tile([128, 512], FR, tag="scp")
                            for t in range(t0, TT):
                                nc.tensor.transpose(
                                    out=trp[:, ds(128 * t, 128)],
                                    in_=att[t][:, ds(128 * c, 128)],
                                    identity=ident,
                                )
                            aT = apool.tile([128, 512], FR, tag="rbs", bufs=4)
                            nc.any.tensor_copy(out=aT[:, ds(128 * t0, wq_)],
                                               in_=trp[:, ds(128 * t0, wq_)])
                            nc.tensor.matmul(
                                out=psc[:64, ds(128 * t0, wq_)],
                                lhsT=VV[c][:, ds(64 * h, 64)],
                                rhs=aT[:, ds(128 * t0, wq_)],
                                start=(c == 0), stop=(c == TT - 1),
                            )
                        ct = apool.tile([64, 512], FR, tag="ctxh", bufs=10)
                        if b_ap is not None:
                            nc.scalar.activation(out=ct, in_=psc[:64, :],
                                                 func=AF.Identity,
                                                 bias=bias_col(b_ap[2, ds(64 * h, 64)]))
                        else:
                            nc.scalar.copy(out=ct, in_=psc[:64, :])
                        ctxT.append(ct)

                z_sd = []
                for t in range(TT):
                    pso = pspool.tile([128, 512], FP, tag="mm")
                    for h in range(H):
                        nc.tensor.matmul(
                            out=pso, lhsT=ctxT[h][:, ts(t, 128)],
                            rhs=wo[h][:64, :],
                            start=(h == 0), stop=(h == H - 1),
                        )
                    z_sd.append(residual(x_sd[t], pso, badd_ap))
                return z_sd

            def mha_any(x_sd, xq_ds, xkv_ds, w_ap, b_ap, causal, kmask,
                        badd_ap=None):
                use_masked = (b_ap is not None or kmask is not None
                              or (causal and flags.get("tgt_mask")))
                if use_masked:
                    return mha_masked(x_sd, xq_ds, xkv_ds, w_ap, b_ap, causal,
                                      kmask, badd_ap)
                return mha(x_sd, xq_ds, xkv_ds, w_ap, causal, badd_ap)

            # ---- FFN --------------------------------------------------------
            def ffn(x_sd, x_ds, w1_ap, w2_ap, b1_ap, b2_ap):
                w1 = [[wtile(w1_ap[ts(c, 128), ts(g2, 512)]) for g2 in range(4)]
                      for c in range(DC)]
                w2 = [wtile(w2_ap[ts(dt, 128), :]) for dt in range(FTL)]
                hT = []
                for dt in range(FTL):
                    g2, r = dt // 4, dt % 4
                    psh = pspool.tile([128, 512], FP, tag="mm")
                    for c in range(DC):
                        nc.tensor.matmul(
                            out=psh, lhsT=w1[c][g2][:, ds(128 * r, 128)], rhs=x_ds[c],
                            start=(c == 0), stop=(c == DC - 1),
                        )
                    ht = hpool.tile([128, 512], FR, tag="hT")
                    if b1_ap is not None:
                        nc.scalar.activation(out=ht, in_=psh, func=AF.Relu,
                                             bias=bias_col(b1_ap[ts(dt, 128)]))
                    else:
                        nc.scalar.activation(out=ht, in_=psh, func=AF.Relu)
                    hT.append(ht)
                z_sd = []
                for t in range(TT):
                    psf = pspool.tile([128, 512], FP, tag="mm")
                    for dt in range(FTL):
                        nc.tensor.matmul(
                            out=psf, lhsT=hT[dt][:, ts(t, 128)], rhs=w2[dt],
                            start=(dt == 0), stop=(dt == FTL - 1),
                        )
                    z_sd.append(residual(x_sd[t], psf, b2_ap))
                return z_sd

            # ================= encoder =================
            x_sd = []
            for t in range(TT):
                xt_ = apool.tile([128, 512], FR, tag="xn", bufs=8)
                nc.sync.dma_start(out=xt_, in_=x0e[ts(t, 128), :])
                x_sd.append(xt_)
            x_ds = transpose_sd(x_sd, "xT")
            if debug_stage == "x0":
                dump_and_stop(x_sd + x_ds)

            for i in range(LE):
                ab = be_attn[i] if flags.get("attn_bias") else None
                bo = be_attn[i, 3] if flags.get("ffn_bias") else None
                z = mha_any(x_sd, x_ds, x_ds, we_attn[i], ab, False, bcast, bo)
                lg = eln_g[i, 0] if flags.get("ln_affine") else None
                lb = eln_b[i, 0] if flags.get("ln_affine") else None
                x_sd, x_ds = ln_block(z, "xT", lg, lb)
                if debug_stage == f"enc{i}.ln1":
                    dump_and_stop(x_sd + x_ds)
                b1 = be_f1[i] if flags.get("ffn_bias") else None
                b2 = be_f2[i] if flags.get("ffn_bias") else None
                z = ffn(x_sd, x_ds, we_f1[i], we_f2[i], b1, b2)
                lg = eln_g[i, 1] if flags.get("ln_affine") else None
                lb = eln_b[i, 1] if flags.get("ln_affine") else None
                x_sd, x_ds = ln_block(z, "xT", lg, lb)
                if debug_stage == f"enc{i}":
                    dump_and_stop(x_sd + x_ds)

            # persist encoder output (transposed) for cross attention
            encT = []
            for c in range(DC):
                e = apool.tile([128, 512], FR, tag="encT", bufs=4)
                nc.any.tensor_copy(out=e, in_=x_ds[c])
                encT.append(e)

            # ================= decoder =================
            y_sd = []
            for t in range(TT):
                yt_ = apool.tile([128, 512], FR, tag="xn", bufs=8)
                nc.sync.dma_start(out=yt_, in_=x0d[ts(t, 128), :])
                y_sd.append(yt_)
            y_ds = transpose_sd(y_sd, "xT")

            for i in range(LD):
                ab = bd_sa[i] if flags.get("attn_bias") else None
                bo = bd_sa[i, 3] if flags.get("ffn_bias") else None
                z = mha_any(y_sd, y_ds, y_ds, wd_sa[i], ab, True, None, bo)
                lg = dln_g[i, 0] if flags.get("ln_affine") else None
                lb = dln_b[i, 0] if flags.get("ln_affine") else None
                y_sd, y_ds = ln_block(z, "xT", lg, lb)

                ab = bd_ca[i] if flags.get("attn_bias") else None
                bo = bd_ca[i, 3] if flags.get("ffn_bias") else None
                z = mha_any(y_sd, y_ds, encT, wd_ca[i], ab, False, bcast, bo)
                lg = dln_g[i, 1] if flags.get("ln_affine") else None
                lb = dln_b[i, 1] if flags.get("ln_affine") else None
                y_sd, y_ds = ln_block(z, "xT", lg, lb)

                b1 = bd_f1[i] if flags.get("ffn_bias") else None
                b2 = bd_f2[i] if flags.get("ffn_bias") else None
                z = ffn(y_sd, y_ds, wd_f1[i], wd_f2[i], b1, b2)
                lg = dln_g[i, 2] if flags.get("ln_affine") else None
                lb = dln_b[i, 2] if flags.get("ln_affine") else None
                y_sd, y_ds = ln_block(z, "xT", lg, lb)
                if debug_stage == f"dec{i}":
                    dump_and_stop(y_sd + y_ds)

            # ================= output projection =================
            nvc = (VH + 511) // 512
            for vc in range(nvc):
                w_ = min(512, VH - 512 * vc)
                wo_c = []
                for c in range(DC):
                    wt_ = wpool.tile([128, 512], FR, tag="w")
                    nc.sync.dma_start(out=wt_[:, :w_],
                                      in_=wout[ts(c, 128), ds(512 * vc, w_)])
                    wo_c.append(wt_)
                for t in range(TT):
                    pso = pspool.tile([128, 512], FP, tag="mm")
                    for c in range(DC):
                        nc.tensor.matmul(
                            out=pso[:, :w_], lhsT=y_ds[c][:, ts(t, 128)],
                            rhs=wo_c[c][:, :w_],
                            start=(c == 0), stop=(c == DC - 1),
                        )
                    sb = apool.tile([128, 512], FP, tag="ob", bufs=4)
                    if flags.get("out_bias"):
                        ob = bcast_tile(bout[ds(512 * vc, w_)], w_)
                        nc.vector.tensor_add(sb[:, :w_], pso[:, :w_], ob)
                    elif (vc * TT + t) % 2 == 0:
                        # alternate PSUM drains across ACT/DVE so the two
                        # mm slots recycle twice as fast in the logits tail
                        nc.scalar.copy(out=sb[:, :w_], in_=pso[:, :w_])
                    else:
                        nc.vector.tensor_copy(out=sb[:, :w_], in_=pso[:, :w_])
                    nc.sync.dma_start(out=logits[ts(t, 128), ds(512 * vc, w_)],
                                      in_=sb[:, :w_])
      except _StopTrace:
          pass

    nc.finalize()
    return nc


def _build_fast(debug_stage=None):
    """v2 builder: no masks/biases/affine-LN.  Single transposed activation
    layout xT:[D,S] (4 chunks of [128,512]).  No PE transposes anywhere:
      - LN stats via ones-column matmuls into one PSUM accumulation group
        ([2,512]: row0 = -mean, row1 = E[z^2]); scale/shift row broadcast via
        gpsimd; apply = 2 DVE passes per chunk.
      - attention out-proj and FFN w2 produce transposed outputs directly
        (lhsT = weight chunk, rhs = ctx/h in dim-partition layout), K=128.
      - decoder-causal scores packed two heads per PE pass via row-group
        tile_position (lhsT base partitions 0/64).
      - logits tail reuses the y stationary across 8 vocab chunks resident
        in all 8 PSUM banks.
    """
    nc = bacc.Bacc(None)

    x0eT = nc.declare_dram_parameter("x0eT", [D, S], FR, isOutput=False)
    x0dT = nc.declare_dram_parameter("x0dT", [D, S], FR, isOutput=False)
    we_attn = nc.declare_dram_parameter("we_attn", [LE, 4, D, D], FR, isOutput=False)
    we_f1 = nc.declare_dram_parameter("we_f1", [LE, D, DFF], FR, isOutput=False)
    we_f2 = nc.declare_dram_parameter("we_f2", [LE, DFF, D], FR, isOutput=False)
    wd_sa = nc.declare_dram_parameter("wd_sa", [LD, 4, D, D], FR, isOutput=False)
    wd_ca = nc.declare_dram_parameter("wd_ca", [LD, 4, D, D], FR, isOutput=False)
    wd_f1 = nc.declare_dram_parameter("wd_f1", [LD, D, DFF], FR, isOutput=False)
    wd_f2 = nc.declare_dram_parameter("wd_f2", [LD, DFF, D], FR, isOutput=False)
    wout = nc.declare_dram_parameter("wout", [D, VH], FR, isOutput=False)
    ctri = nc.declare_dram_parameter("ctri", [128, 128], FR, isOutput=False)
    cstat = nc.declare_dram_parameter("cstat", [128, 256], FR, isOutput=False)
    logits = nc.declare_dram_parameter("logits", [S, VH], FR, isOutput=True)
    dbg = None
    if debug_stage is not None:
        dbg = nc.declare_dram_parameter("dbg", [8, 128, 512], FP, isOutput=True)

    from contextlib import ExitStack

    with tile.TileContext(nc) as tc, ExitStack() as stk:
      try:
        wpool = stk.enter_context(tc.tile_pool(name="w", bufs=72))
        apool = stk.enter_context(tc.tile_pool(name="acts", bufs=2))
        hpool = stk.enter_context(tc.tile_pool(name="h", bufs=16))
        cpool = stk.enter_context(tc.tile_pool(name="consts", bufs=1))
        spool = stk.enter_context(tc.tile_pool(name="small", bufs=12))
        pspool = stk.enter_context(tc.tile_pool(name="ps", bufs=2, space="PSUM"))
        scpool = stk.enter_context(tc.tile_pool(name="sc", bufs=2, space="PSUM"))
        pcpool = stk.enter_context(tc.tile_pool(name="pc", bufs=2, space="PSUM"))
        stpool = stk.enter_context(tc.tile_pool(name="st", bufs=2, space="PSUM"))
        if True:
            # ---- constants --------------------------------------------------
            tri = cpool.tile([128, 128], FR)
            nc.sync.dma_start(out=tri, in_=ctri[:, :])
            statw = cpool.tile([128, 256], FR)
            nc.sync.dma_start(out=statw, in_=cstat[:, :])
            eps = cpool.tile([128, 1], FP)
            nc.vector.memset(eps, 1e-5)

            junk = cpool.tile([128, 1], FP, tag="junk")
            nc.vector.memset(junk, 1.0)

            def act_warm(func):
                # dummy activation to pull the ACT function table in while
                # the PE is busy, so the real op doesn't pay the ~1.3us
                # table reload on the LN critical path
                j = spool.tile([128, 1], FP, tag="jk", bufs=2)
                nc.scalar.activation(out=j, in_=junk, func=func)

            def wtile(dram_ap, tag="w"):
                t_ = wpool.tile([128, 512], FR, tag=tag)
                nc.sync.dma_start(out=t_, in_=dram_ap)
                return t_

            def dump_and_stop(tiles):
                for j, t_ in enumerate(tiles[:8]):
                    stt = apool.tile([128, 512], FP, tag="ob", bufs=4)
                    pp, ff = t_.shape[-2], t_.shape[-1]
                    nc.any.tensor_copy(out=stt[:pp, :ff], in_=t_)
                    nc.sync.dma_start(out=dbg[j, :pp, :ff], in_=stt[:pp, :ff])
                raise _StopTrace

            # ---- layernorm over the partition (D) axis ----------------------
            # z chunks are [128,512] bf16 (dims x tokens).  Stats:
            #   pstat[0,:] = sum_c (-1/512) * z_c   (= -mean)
            #   pstat[1,:] = sum_c (1/512) * z_c^2  (= E[z^2])
            # one PSUM accumulation group; two stationaries (cstat cols).
            def ln_T(z_ds, out_tag, out_bufs=8):
                # stats matmuls use a [128,128] constant stationary (+-1/512),
                # so every output partition carries the same row: the stats
                # arrive pre-broadcast and no gpsimd partition_broadcast is
                # needed anywhere in the chain.
                act_warm(AF.Abs_reciprocal_sqrt)
                pstatA = stpool.tile([128, 512], FP, tag="pstat")  # -mean
                pstatB = stpool.tile([128, 512], FP, tag="pstat")  # E[z^2]
                zsq = []
                for c in range(DC):
                    zq = apool.tile([128, 512], FR, tag="zsq", bufs=4)
                    nc.vector.tensor_mul(zq, z_ds[c], z_ds[c])
                    zsq.append(zq)
                for c in range(DC):
                    nc.tensor.matmul(
                        out=pstatA, lhsT=statw[:, 0:128], rhs=z_ds[c],
                        start=(c == 0), stop=(c == DC - 1),
                    )
                for c in range(DC):
                    nc.tensor.matmul(
                        out=pstatB, lhsT=statw[:, 128:256], rhs=zsq[c],
                        start=(c == 0), stop=(c == DC - 1),
                    )
                # centered z (zc = z - mu) runs off pstatA while the PE does
                # the sq stats matmuls; only the final scale multiply waits on
                # the rsqrt.
                stage = apool.tile([128, 512], FP, tag="stg", bufs=2)
                nc.vector.tensor_copy(out=stage, in_=pstatA)
                musq = apool.tile([128, 512], FP, tag="musq", bufs=2)
                nc.vector.tensor_mul(musq, stage, stage)
                var = apool.tile([128, 512], FP, tag="var", bufs=2)
                nc.vector.tensor_sub(var, pstatB, musq)
                bcA = apool.tile([128, 512], FR, tag="lnbcA", bufs=2)
                nc.scalar.activation(out=bcA, in_=var,
                                     func=AF.Abs_reciprocal_sqrt, bias=eps)
                zc_ds = []
                for c in range(DC):
                    zc = apool.tile([128, 512], FR, tag="zc", bufs=4)
                    nc.vector.tensor_add(zc, z_ds[c], pstatA)
                    zc_ds.append(zc)
                xn_ds = []
                for c in range(DC):
                    xn = apool.tile([128, 512], FR, tag=out_tag, bufs=out_bufs)
                    nc.vector.tensor_mul(xn, zc_ds[c], bcA)
                    xn_ds.append(xn)
                return xn_ds

            # ---- multi-head attention (transposed everything) ---------------
            # persistent V ring: per head 64 V-dim cols + 64 static ones cols
            # ([128, 8*128]); the ones make the ctx matmul emit the softmax
            # denominator pre-broadcast on psum partitions 64-127.
            NVR = 6
            vvo_ring = [apool.tile([128, 1024], FR, tag="vvo", bufs=NVR,
                                   name=f"vvo{i}") for i in range(NVR)]
            for i in range(NVR):
                nc.vector.memset(
                    _view3(vvo_ring[i][:, :], 8, 128, 64, inner_off=64), 1.0)
            vv_ctr = [0]
            mha_ctr = [0]

            def mha_T(xqT, xkvT, w_ap, causal):
                """Returns 4 z chunks [128,512] bf16 in [D,S] layout
                (attention output + residual xqT)."""
                midx = mha_ctr[0]; mha_ctr[0] += 1
                act_warm(AF.Exp)
                wq = [wtile(w_ap[0, ts(c, 128), :]) for c in range(DC)]
                wk = [wtile(w_ap[1, ts(c, 128), :]) for c in range(DC)]
                wv = [wtile(w_ap[2, ts(c, 128), :]) for c in range(DC)]
                wo = [wtile(w_ap[3, ts(p, 128), :]) for p in range(G)]

                QT, KT = [], []
                for g in range(G):
                    psq = pspool.tile([128, 512], FP, tag="mm")
                    for c in range(DC):
                        nc.tensor.matmul(
                            out=psq, lhsT=wq[c][:, ts(g, 128)], rhs=xqT[c],
                            start=(c == 0), stop=(c == DC - 1),
                        )
                    qt = apool.tile([128, 512], FR, tag="qt", bufs=5)
                    nc.vector.tensor_copy(out=qt, in_=psq)
                    QT.append(qt)
                    psk = pspool.tile([128, 512], FP, tag="mm")
                    for c in range(DC):
                        nc.tensor.matmul(
                            out=psk, lhsT=wk[c][:, ts(g, 128)], rhs=xkvT[c],
                            start=(c == 0), stop=(c == DC - 1),
                        )
                    kt = apool.tile([128, 512], FR, tag="kt", bufs=5)
                    nc.vector.tensor_copy(out=kt, in_=psk)
                    KT.append(kt)

                # V ring tiles: write the 8x64 V views; ones cols persist
                VO = []
                for t in range(TT):
                    psv = pspool.tile([128, 512], FP, tag="mm")
                    for c in range(DC):
                        nc.tensor.matmul(
                            out=psv, lhsT=xkvT[c][:, ts(t, 128)], rhs=wv[c],
                            start=(c == 0), stop=(c == DC - 1),
                        )
                    vvo = vvo_ring[vv_ctr[0] % NVR]
                    vv_ctr[0] += 1
                    nc.vector.tensor_copy(
                        out=_view3(vvo[:, :], 8, 128, 64),
                        in_=_view3(psv[:, :], 8, 64, 64),
                    )
                    VO.append(vvo)
                if debug_stage == f"mha{midx}.qkv":
                    dump_and_stop(QT + KT)
                if debug_stage == f"mha{midx}.v":
                    dump_and_stop(VO)

                ctxP = [apool.tile([128, 512], FR, tag="ctxP", bufs=6,
                                   name=f"ctxP{p}") for p in range(G)]
                for g in range(G):
                    # scores for the two heads of this pair run concurrently
                    # on PE row-groups 0-1 / 2-3 (lhsT base partition 0 / 64).
                    ES = [[], []]
                    for kt in range(TT):
                        q0 = 128 * kt if causal else 0
                        W = 512 - q0
                        scps = []
                        for lh in range(2):
                            qsl = slice(64 * lh, 64 * lh + 64)
                            scp = scpool.tile([128, 512], FP, tag="scp")
                            nc.tensor.matmul(
                                out=scp[:, ds(q0, W)],
                                lhsT=KT[g][qsl, ts(kt, 128)],
                                rhs=QT[g][qsl, ds(q0, W)],
                                start=True, stop=True,
                            )
                            scps.append(scp)
                        for lh in range(2):
                            scp = scps[lh]
                            if causal:
                                nc.vector.tensor_add(
                                    scp[:, ds(q0, 128)], scp[:, ds(q0, 128)], tri)
                            es = apool.tile([128, 512], FR, tag="exp", bufs=10)
                            nc.scalar.activation(out=es[:, ds(q0, W)],
                                                 in_=scp[:, ds(q0, W)], func=AF.Exp)
                            ES[lh].append(es)
                    for lh in range(2):
                        h = 2 * g + lh
                        psc = pcpool.tile([128, 512], FP, tag="psc")
                        for kt in range(TT):
                            q0 = 128 * kt if causal else 0
                            W = 512 - q0
                            nc.tensor.matmul(
                                out=psc[:, ds(q0, W)],
                                lhsT=VO[kt][:, ds(128 * h, 128)],
                                rhs=ES[lh][kt][:, ds(q0, W)],
                                start=(kt == 0), stop=(kt == TT - 1),
                            )
                        den = spool.tile([64, 512], FP, tag="den", bufs=4)
                        nc.vector.tensor_copy(out=den, in_=psc[64:128, :])
                        rec = spool.tile([64, 512], FP, tag="rec", bufs=4)
                        nc.vector.reciprocal_approx_fast(out=rec, in_=den)
                        nc.vector.tensor_mul(
                            ctxP[g][64 * lh: 64 * lh + 64, :],
                            psc[:64, :], rec)

                if debug_stage == f"mha{midx}.ctx":
                    dump_and_stop(ctxP)
                z_ds = []
                for c in range(DC):
                    pso = pspool.tile([128, 512], FP, tag="mm")
                    for p in range(G):
                        nc.tensor.matmul(
                            out=pso, lhsT=wo[p][:, ts(c, 128)], rhs=ctxP[p],
                            start=(p == 0), stop=(p == G - 1),
                        )
                    z = apool.tile([128, 512], FR, tag="z", bufs=4)
                    nc.vector.tensor_add(z, xqT[c], pso)
                    z_ds.append(z)
                if debug_stage == f"mha{midx}.out":
                    dump_and_stop(z_ds)
                return z_ds

            # ---- FFN (transposed output) ------------------------------------
            def ffn_T(xT, w1_ap, w2_ap):
                act_warm(AF.Relu)
                w1 = [[wtile(w1_ap[ts(c, 128), ts(g2, 512)]) for g2 in range(4)]
                      for c in range(DC)]
                w2 = [wtile(w2_ap[ts(dt, 128), :]) for dt in range(FTL)]
                hT = []
                for dt in range(FTL):
                    g2, r = dt // 4, dt % 4
                    psh = pspool.tile([128, 512], FP, tag="mm")
                    for c in range(DC):
                        nc.tensor.matmul(
                            out=psh, lhsT=w1[c][g2][:, ds(128 * r, 128)], rhs=xT[c],
                            start=(c == 0), stop=(c == DC - 1),
                        )
                    ht = hpool.tile([128, 512], FR, tag="hT")
                    nc.scalar.activation(out=ht, in_=psh, func=AF.Relu)
                    hT.append(ht)
                z_ds = []
                for c in range(DC):
                    psf = pspool.tile([128, 512], FP, tag="mm")
                    for dt in range(FTL):
                        nc.tensor.matmul(
                            out=psf, lhsT=w2[dt][:, ts(c, 128)], rhs=hT[dt],
                            start=(dt == 0), stop=(dt == FTL - 1),
                        )
                    z = apool.tile([128, 512], FR, tag="z", bufs=4)
                    nc.vector.tensor_add(z, xT[c], psf)
                    z_ds.append(z)
                return z_ds

            # ================= encoder =================
            x_ds = []
            for c in range(DC):
                xt_ = apool.tile([128, 512], FR, tag="xn", bufs=8)
                nc.sync.dma_start(out=xt_, in_=x0eT[ts(c, 128), :])
                x_ds.append(xt_)
            if debug_stage == "x0":
                dump_and_stop(x_ds)

            for i in range(LE):
                z = mha_T(x_ds, x_ds, we_attn[i], False)
                x_ds = ln_T(z, "xn")
                if debug_stage == f"enc{i}.ln1":
                    dump_and_stop(x_ds)
                z = ffn_T(x_ds, we_f1[i], we_f2[i])
                out_tag = "encT" if i == LE - 1 else "xn"
                x_ds = ln_T(z, out_tag, out_bufs=4 if i == LE - 1 else 8)
                if debug_stage == f"enc{i}":
                    dump_and_stop(x_ds)
            encT = x_ds

            # ================= decoder =================
            y_ds = []
            for c in range(DC):
                yt_ = apool.tile([128, 512], FR, tag="xn", bufs=8)
                nc.sync.dma_start(out=yt_, in_=x0dT[ts(c, 128), :])
                y_ds.append(yt_)

            for i in range(LD):
                z = mha_T(y_ds, y_ds, wd_sa[i], True)
                y_ds = ln_T(z, "xn")
                z = mha_T(y_ds, encT, wd_ca[i], False)
                y_ds = ln_T(z, "xn")
                z = ffn_T(y_ds, wd_f1[i], wd_f2[i])
                out_tag = "y" if i == LD - 1 else "xn"
                y_ds = ln_T(z, out_tag, out_bufs=4 if i == LD - 1 else 8)
                if debug_stage == f"dec{i}":
                    dump_and_stop(y_ds)

            # ================= output projection =================
            # 32 vocab chunks of <=512, in groups of 8 (one PSUM bank each);
            # stationary y[c][:, t-block] is reused across the 8 chunks.
            nvc = (VH + 511) // 512
            psum_of = [pspool, pspool, scpool, scpool,
                       pcpool, pcpool, stpool, stpool]
            ptag = ["mm", "mm", "scp", "scp", "psc", "psc", "pstat", "pstat"]
            for vg in range(0, nvc, 8):
                grp = list(range(vg, min(vg + 8, nvc)))
                wt_g = {}
                for c in range(DC):
                    for j in grp:
                        w_ = min(512, VH - 512 * j)
                        wt_ = wpool.tile([128, 512], FR, tag="w")
                        nc.sync.dma_start(out=wt_[:, :w_],
                                          in_=wout[ts(c, 128), ds(512 * j, w_)])
                        wt_g[(c, j)] = wt_
                for t in range(TT):
                    pss = [psum_of[k].tile([128, 512], FP, tag=ptag[k],
                                           name=f"pl{k}") for k in range(len(grp))]
                    for c in range(DC):
                        for k, j in enumerate(grp):
                            w_ = min(512, VH - 512 * j)
                            nc.tensor.matmul(
                                out=pss[k][:, :w_],
                                lhsT=y_ds[c][:, ts(t, 128)],
                                rhs=wt_g[(c, j)][:, :w_],
                                start=(c == 0), stop=(c == DC - 1),
                            )
                    for k, j in enumerate(grp):
                        w_ = min(512, VH - 512 * j)
                        sb = apool.tile([128, 512], FR, tag="ob", bufs=8)
                        if k % 2 == 0:
                            nc.scalar.copy(out=sb[:, :w_], in_=pss[k][:, :w_])
                        else:
                            nc.vector.tensor_copy(out=sb[:, :w_], in_=pss[k][:, :w_])
                        nc.sync.dma_start(out=logits[ts(t, 128), ds(512 * j, w_)],
                                          in_=sb[:, :w_])
      except _StopTrace:
          pass

    nc.finalize()
    return nc


def _host_prep(inputs):
    """Host-side preparation: embeddings, weight folding, masks, flags."""
    src = np.asarray(inputs["src"])
    tgt = np.asarray(inputs["tgt"])
    f32 = lambda k: np.ascontiguousarray(np.asarray(inputs[k], dtype=np.float32))

    enc_emb, dec_emb, pe = f32("enc_emb"), f32("dec_emb"), f32("pe")
    sqd = np.float32(math.sqrt(D))
    x0e = enc_emb[src] * sqd + pe[None, :S]          # [B, S, D]
    x0d = dec_emb[tgt] * sqd + pe[None, :S]

    we_attn = f32("enc_attn_w").copy()
    wd_sa = f32("dec_sa_w").copy()
    wd_ca = f32("dec_ca_w").copy()
    scale = np.float32(1.0 / math.sqrt(DK))
    we_attn[:, 0] *= scale
    wd_sa[:, 0] *= scale
    wd_ca[:, 0] *= scale
    be_attn = f32("enc_attn_b").copy()
    bd_sa = f32("dec_sa_b").copy()
    bd_ca = f32("dec_ca_b").copy()
    be_attn[:, 0] *= scale
    bd_sa[:, 0] *= scale
    bd_ca[:, 0] *= scale

    flags = {
        "attn_bias": bool(np.any(be_attn[:, (0, 2)]) or np.any(bd_sa[:, (0, 2)])
                          or np.any(bd_ca[:, (0, 2)])),
        "ffn_bias": bool(np.any(f32("enc_ffn_b1")) or np.any(f32("enc_ffn_b2"))
                         or np.any(f32("dec_ffn_b1")) or np.any(f32("dec_ffn_b2"))
                         or np.any(be_attn[:, 3]) or np.any(bd_sa[:, 3])
                         or np.any(bd_ca[:, 3])),
        "ln_affine": bool(np.any(f32("enc_ln_g") != 1.0) or np.any(f32("enc_ln_b"))
                          or np.any(f32("dec_ln_g") != 1.0) or np.any(f32("dec_ln_b"))),
        "out_bias": bool(np.any(f32("out_b"))),
        "src_mask": bool((src == 0).any()),
        "tgt_mask": bool((tgt == 0).any()),
    }

    bf = lambda a: np.ascontiguousarray(np.asarray(a, np.float32).astype(BFNP))
    cident = np.eye(128, dtype=np.float32)
    ctri = np.tril(np.full((128, 128), NEG, dtype=np.float32), k=-1)

    common = {
        "we_attn": bf(we_attn),
        "we_f1": bf(f32("enc_ffn_w1")), "we_f2": bf(f32("enc_ffn_w2")),
        "wd_sa": bf(wd_sa), "wd_ca": bf(wd_ca),
        "wd_f1": bf(f32("dec_ffn_w1")), "wd_f2": bf(f32("dec_ffn_w2")),
        "cident": bf(cident), "ctri": bf(ctri),
        "cones": np.ones((1, 64), np.float32),
    }
    if flags["attn_bias"]:
        common.update(be_attn=np.ascontiguousarray(be_attn),
                      bd_sa=np.ascontiguousarray(bd_sa),
                      bd_ca=np.ascontiguousarray(bd_ca))
    if flags["ffn_bias"]:
        common.update(be_f1=f32("enc_ffn_b1"), be_f2=f32("enc_ffn_b2"),
                      bd_f1=f32("dec_ffn_b1"), bd_f2=f32("dec_ffn_b2"))
    if flags["ln_affine"]:
        common.update(eln_g=f32("enc_ln_g"), eln_b=f32("enc_ln_b"),
                      dln_g=f32("dec_ln_g"), dln_b=f32("dec_ln_b"))

    out_w = f32("out_w")
    out_b = f32("out_b")

    in_maps = []
    for core in range(NCORES):
        b, half = core // 2, core % 2
        m = dict(common)
        m["x0e"] = bf(x0e[b])
        m["x0d"] = bf(x0d[b])
        m["wout"] = bf(out_w[:, half * VH : (half + 1) * VH])
        if flags["out_bias"]:
            m["bout"] = np.ascontiguousarray(out_b[half * VH : (half + 1) * VH])
        if flags["src_mask"]:
            m["km_src"] = np.where(src[b] != 0, 0.0, NEG).astype(np.float32)
        if flags["tgt_mask"]:
            rm = (tgt[b] != 0).astype(np.float32)
            m["rm_tgt"] = rm
            cm = np.where(np.tril(np.ones((S, S), bool)), 0.0, NEG).astype(np.float32)
            cm = cm * rm[:, None]          # padded query rows -> all-zero scores
            m["cm_tgt"] = np.ascontiguousarray(cm)
        in_maps.append(m)

    return flags, in_maps


def _any_special(inputs):
    """True if any mask/bias/affine feature is active (v1 fallback needed)."""
    f32 = lambda k: np.asarray(inputs[k], dtype=np.float32)
    src = np.asarray(inputs["src"])
    tgt = np.asarray(inputs["tgt"])
    return bool(
        np.any(f32("enc_attn_b")) or np.any(f32("dec_sa_b"))
        or np.any(f32("dec_ca_b"))
        or np.any(f32("enc_ffn_b1")) or np.any(f32("enc_ffn_b2"))
        or np.any(f32("dec_ffn_b1")) or np.any(f32("dec_ffn_b2"))
        or np.any(f32("enc_ln_g") != 1.0) or np.any(f32("enc_ln_b"))
        or np.any(f32("dec_ln_g") != 1.0) or np.any(f32("dec_ln_b"))
        or np.any(f32("out_b"))
        or (src == 0).any() or (tgt == 0).any()
    )


def _host_prep_fast(inputs):
    src = np.asarray(inputs["src"])
    tgt = np.asarray(inputs["tgt"])
    f32 = lambda k: np.ascontiguousarray(np.asarray(inputs[k], dtype=np.float32))

    enc_emb, dec_emb, pe = f32("enc_emb"), f32("dec_emb"), f32("pe")
    sqd = np.float32(math.sqrt(D))
    x0e = enc_emb[src] * sqd + pe[None, :S]          # [B, S, D]
    x0d = dec_emb[tgt] * sqd + pe[None, :S]

    we_attn = f32("enc_attn_w").copy()
    wd_sa = f32("dec_sa_w").copy()
    wd_ca = f32("dec_ca_w").copy()
    scale = np.float32(1.0 / math.sqrt(DK))
    we_attn[:, 0] *= scale
    wd_sa[:, 0] *= scale
    wd_ca[:, 0] *= scale

    bf = lambda a: np.ascontiguousarray(np.asarray(a, np.float32).astype(BFNP))
    ctri = np.tril(np.full((128, 128), NEG, dtype=np.float32), k=-1)
    cstat = np.zeros((128, 256), np.float32)
    cstat[:, 0:128] = -1.0 / 512
    cstat[:, 128:256] = 1.0 / 512

    common = {
        "we_attn": bf(we_attn),
        "we_f1": bf(f32("enc_ffn_w1")), "we_f2": bf(f32("enc_ffn_w2")),
        "wd_sa": bf(wd_sa), "wd_ca": bf(wd_ca),
        "wd_f1": bf(f32("dec_ffn_w1")), "wd_f2": bf(f32("dec_ffn_w2")),
        "ctri": bf(ctri), "cstat": bf(cstat),
    }
    out_w = f32("out_w")
    in_maps = []
    for core in range(NCORES):
        b, half = core // 2, core % 2
        m = dict(common)
        m["x0eT"] = bf(np.ascontiguousarray(x0e[b].T))
        m["x0dT"] = bf(np.ascontiguousarray(x0d[b].T))
        m["wout"] = bf(out_w[:, half * VH: (half + 1) * VH])
        in_maps.append(m)
    return in_maps


def _run(inputs, trace=False, debug_stage=None, **kwargs):
    if not _any_special(inputs):
        in_maps = _host_prep_fast(inputs)
        nc = _build_fast(debug_stage)
        res = run_bass_kernel_spmd(nc, in_maps, list(range(NCORES)),
                                   trace=trace, **kwargs)
        out = np.zeros((4, S, V), dtype=np.float32)
        for core in range(NCORES):
            b, half = core // 2, core % 2
            out[b, :, half * VH: (half + 1) * VH] = np.asarray(
                res.results[core]["logits"], np.float32)
        return out, res
    flags, in_maps = _host_prep(inputs)
    if flags["ffn_bias"] and not flags["attn_bias"]:
        # mha's bo path reads attn-bias tensors; force-declare them
        flags["attn_bias"] = True
        be = np.asarray(inputs["enc_attn_b"], np.float32).copy()
        be[:, 0] *= np.float32(1.0 / math.sqrt(DK))
        bs = np.asarray(inputs["dec_sa_b"], np.float32).copy()
        bs[:, 0] *= np.float32(1.0 / math.sqrt(DK))
        bc = np.asarray(inputs["dec_ca_b"], np.float32).copy()
        bc[:, 0] *= np.float32(1.0 / math.sqrt(DK))
        for m in in_maps:
            m.update(be_attn=np.ascontiguousarray(be),
                     bd_sa=np.ascontiguousarray(bs),
                     bd_ca=np.ascontiguousarray(bc))
    nc = _build_program(flags)
    res = run_bass_kernel_spmd(nc, in_maps, list(range(NCORES)), trace=trace, **kwargs)
    out = np.zeros((4, S, V), dtype=np.float32)
    for core in range(NCORES):
        b, half = core // 2, core % 2
        out[b, :, half * VH : (half + 1) * VH] = res.results[core]["logits"]
    return out, res


def kernel(**inputs):
    out, _ = _run(inputs, trace=False)
    return out



# revision 23
# speedup vs baseline: 1.0212x; 1.0014x over previous
"""Trainium2 Bass kernel for nn_CustomTransformer_64570538328578.

Encoder-decoder transformer: V=32000, D=512, H=8, L=6+6, DFF=2048, B=4, S=512.

Sharding: 8 cores = 4 batch pairs x 2 vocab halves.  Core c handles batch
element c//2 (full encoder+decoder stack, duplicated within the pair) and
computes logits for vocab half c%2 of the output projection.  No on-device
collectives needed.

All matmul data is bf16 (PSUM accumulation fp32).  Attention is computed in
transposed-score layout: ST[k,q] = (K^T)^T-by-Q products per k-tile, exp on
ACT, and the softmax denominator comes for free from a ones-column appended
to the V stationary (65-row ctx matmul).  The per-query reciprocal is
broadcast across partitions with a rank-1 fp32r matmul, so no per-head
transposes of the attention matrix are needed at all.

Layouts on device (per core):
  - canonical activations x: [S, D] as 4 tiles [128, 512] (token-partition)
  - transposed activations xT: [D, S] as 4 tiles [128, 512] (dim-partition)
  - per-head QT/KT: [DK, S] packed 2 heads/tile -> 4 tiles [128, 512]
  - V+ones: [S, 8*65] 4 tiles (per-head 64 dims + ones col side by side)
  - scoresT per (head, k-tile): PSUM [128, <=512]; causal diag mask added
    via an ident x tril(-1e9) matmul into the first 128 columns.
"""

import math
import sys

import ml_dtypes
import numpy as np

if "/opt/trn_rl_repo" not in sys.path:
    sys.path.insert(0, "/opt/trn_rl_repo")

import concourse.bass as bass
import concourse.tile as tile
from concourse import bacc
from concourse import mybir
from concourse.bass import ds, ts
from concourse.bass_utils import run_bass_kernel_spmd

FP = mybir.dt.float32
F32R = mybir.dt.float32r
S = 512
D = 512
H = 8
DK = 64
DFF = 2048
LE = 6
LD = 6
V = 32000
NCORES = 8
VH = V // 2        # vocab half per core
TT = 4             # token tiles (S / 128)
DC = 4             # D chunks of 128
G = 4              # head-pair groups (2 heads of 64 dims -> 128 partitions)
FTL = 16           # dff tiles of 128
NEG = -1.0e9
AF = mybir.ActivationFunctionType
FR = mybir.dt.bfloat16   # matmul/activation storage dtype
BFNP = ml_dtypes.bfloat16


def _f32r(ap):
    return ap.bitcast(F32R)


def _view3(ap, groups, gstride, inner, inner_off=0):
    """[128, x] AP -> [128, groups, inner] view with group stride gstride."""
    a = ap
    return bass.AP(
        tensor=a.tensor,
        offset=a.offset + inner_off,
        ap=[a.ap[0], [gstride, groups], [1, inner]],
    )


class _StopTrace(Exception):
    pass


def _build_program(flags, debug_stage=None):
    """Build the single SPMD Bass program (same for all cores)."""
    nc = bacc.Bacc(None)

    # ---- DRAM parameters ----------------------------------------------------
    x0e = nc.declare_dram_parameter("x0e", [S, D], FR, isOutput=False)
    x0d = nc.declare_dram_parameter("x0d", [S, D], FR, isOutput=False)
    we_attn = nc.declare_dram_parameter("we_attn", [LE, 4, D, D], FR, isOutput=False)
    we_f1 = nc.declare_dram_parameter("we_f1", [LE, D, DFF], FR, isOutput=False)
    we_f2 = nc.declare_dram_parameter("we_f2", [LE, DFF, D], FR, isOutput=False)
    wd_sa = nc.declare_dram_parameter("wd_sa", [LD, 4, D, D], FR, isOutput=False)
    wd_ca = nc.declare_dram_parameter("wd_ca", [LD, 4, D, D], FR, isOutput=False)
    wd_f1 = nc.declare_dram_parameter("wd_f1", [LD, D, DFF], FR, isOutput=False)
    wd_f2 = nc.declare_dram_parameter("wd_f2", [LD, DFF, D], FR, isOutput=False)
    wout = nc.declare_dram_parameter("wout", [D, VH], FR, isOutput=False)
    cident = nc.declare_dram_parameter("cident", [128, 128], FR, isOutput=False)
    ctri = nc.declare_dram_parameter("ctri", [128, 128], FR, isOutput=False)
    cones = nc.declare_dram_parameter("cones", [1, 64], F32R, isOutput=False)
    logits = nc.declare_dram_parameter("logits", [S, VH], FP, isOutput=True)
    dbg = None
    if debug_stage is not None:
        dbg = nc.declare_dram_parameter("dbg", [8, 128, 512], FP, isOutput=True)

    # optional (general-path) params, declared only when actually used
    if flags.get("attn_bias"):
        be_attn = nc.declare_dram_parameter("be_attn", [LE, 4, D], FP, isOutput=False)
        bd_sa = nc.declare_dram_parameter("bd_sa", [LD, 4, D], FP, isOutput=False)
        bd_ca = nc.declare_dram_parameter("bd_ca", [LD, 4, D], FP, isOutput=False)
    if flags.get("ffn_bias"):
        be_f1 = nc.declare_dram_parameter("be_f1", [LE, DFF], FP, isOutput=False)
        be_f2 = nc.declare_dram_parameter("be_f2", [LE, D], FP, isOutput=False)
        bd_f1 = nc.declare_dram_parameter("bd_f1", [LD, DFF], FP, isOutput=False)
        bd_f2 = nc.declare_dram_parameter("bd_f2", [LD, D], FP, isOutput=False)
    if flags.get("ln_affine"):
        eln_g = nc.declare_dram_parameter("eln_g", [LE, 2, D], FP, isOutput=False)
        eln_b = nc.declare_dram_parameter("eln_b", [LE, 2, D], FP, isOutput=False)
        dln_g = nc.declare_dram_parameter("dln_g", [LD, 3, D], FP, isOutput=False)
        dln_b = nc.declare_dram_parameter("dln_b", [LD, 3, D], FP, isOutput=False)
    if flags.get("out_bias"):
        bout = nc.declare_dram_parameter("bout", [VH], FP, isOutput=False)
    if flags.get("src_mask"):
        km_src = nc.declare_dram_parameter("km_src", [S], FP, isOutput=False)
    if flags.get("tgt_mask"):
        rm_tgt = nc.declare_dram_parameter("rm_tgt", [S], FP, isOutput=False)
        cm_tgt = nc.declare_dram_parameter("cm_tgt", [S, S], FP, isOutput=False)

    from contextlib import ExitStack

    with tile.TileContext(nc) as tc, ExitStack() as stk:
      try:
        wpool = stk.enter_context(tc.tile_pool(name="w", bufs=64))
        apool = stk.enter_context(tc.tile_pool(name="acts", bufs=2))
        hpool = stk.enter_context(tc.tile_pool(name="h", bufs=16))
        cpool = stk.enter_context(tc.tile_pool(name="consts", bufs=1))
        spool = stk.enter_context(tc.tile_pool(name="small", bufs=12))
        pspool = stk.enter_context(tc.tile_pool(name="ps", bufs=2, space="PSUM"))
        pcpool = stk.enter_context(tc.tile_pool(name="pc", bufs=3, space="PSUM"))
        scpool = stk.enter_context(tc.tile_pool(name="sc", bufs=3, space="PSUM"))
        if True:

            # ---- constants --------------------------------------------------
            ident = cpool.tile([128, 128], FR)
            nc.sync.dma_start(out=ident, in_=cident[:, :])
            tri = cpool.tile([128, 128], FR)
            nc.sync.dma_start(out=tri, in_=ctri[:, :])
            eps = cpool.tile([128, 1], FP)
            nc.vector.memset(eps, 1e-5)
            ones64 = cpool.tile([1, 64], F32R)
            nc.sync.dma_start(out=ones64, in_=cones[:, :])

            bcast = None
            if flags.get("src_mask"):
                bcast = cpool.tile([128, S], FP)
                kma = km_src[:]
                nc.sync.dma_start(
                    out=bcast,
                    in_=bass.AP(
                        tensor=kma.tensor,
                        offset=kma.offset,
                        ap=[[0, 128]] + kma.ap,
                    ),
                )
            rmt = None
            if flags.get("tgt_mask"):
                rmt = cpool.tile([128, TT], FP)
                for t in range(TT):
                    nc.sync.dma_start(out=rmt[:, t : t + 1], in_=rm_tgt[ts(t, 128)])
                cmt = []
                for t in range(TT):
                    cm = cpool.tile([128, S], FP, tag="cmt")
                    nc.sync.dma_start(out=cm, in_=cm_tgt[ts(t, 128), :])
                    cmt.append(cm)

            junk = cpool.tile([128, 1], FP, tag="junk")
            nc.vector.memset(junk, 1.0)

            def act_warm(func):
                # dummy activation to pull the ACT function table in while
                # the PE is busy, so the real op doesn't pay the ~1.3us
                # table reload on the LN critical path
                j = spool.tile([128, 1], FP, tag="jk", bufs=2)
                nc.scalar.activation(out=j, in_=junk, func=func)

            def wtile(dram_ap, tag="w"):
                t_ = wpool.tile([128, 512], FR, tag=tag)
                rows = dram_ap.shape[-2]
                nc.sync.dma_start(out=t_[:rows, :], in_=dram_ap)
                return t_

            def bias_col(dram_ap):
                n = dram_ap.shape[-1]
                b = spool.tile([128, 1], FP, tag="bias")
                nc.sync.dma_start(out=b[:n, :], in_=dram_ap)
                return b[:n, :]

            def bcast_tile(dram_ap, n):
                """[n] dram vector -> [128, n] sbuf tile (partition broadcast)."""
                b = apool.tile([128, n], FP, tag="bc", bufs=2)
                nc.sync.dma_start(
                    out=b,
                    in_=bass.AP(
                        tensor=dram_ap.tensor,
                        offset=dram_ap.offset,
                        ap=[[0, 128]] + dram_ap.ap,
                    ),
                )
                return b

            def dump_and_stop(tiles):
                for j, t_ in enumerate(tiles[:8]):
                    stt = apool.tile([128, 512], FP, tag="ob", bufs=4)
                    pp, ff = t_.shape[-2], t_.shape[-1]
                    nc.any.tensor_copy(out=stt[:pp, :ff], in_=t_)
                    nc.sync.dma_start(out=dbg[j, :pp, :ff], in_=stt[:pp, :ff])
                raise _StopTrace

            # ---- transpose helper: [S,D] tiles -> [D,S] tiles ---------------
            def transpose_sd(sd_tiles, out_tag):
                ds_tiles = []
                for c in range(DC):
                    trp = scpool.tile([128, 512], FR, tag="scp")
                    for t in range(TT):
                        nc.tensor.transpose(
                            out=trp[:, ds(128 * t, 128)],
                            in_=sd_tiles[t][:, ds(128 * c, 128)],
                            identity=ident,
                        )
                    xt = apool.tile([128, 512], FR, tag=out_tag, bufs=8)
                    # ACT is idle during the LN phase; keep DVE free for stats
                    nc.scalar.copy(out=xt, in_=trp)
                    ds_tiles.append(xt)
                return ds_tiles

            # ---- layernorm (input z already includes the residual) ----------
            def ln_block(z_sd, xt_tag, g_ap=None, b_ap=None):
                new_sd = []
                for t in range(TT):
                    z = z_sd[t]
                    st6 = spool.tile([128, 6], FP, tag="st6")
                    nc.vector.bn_stats(out=st6, in_=z)
                    mv = spool.tile([128, 2], FP, tag="mv")
                    nc.vector.bn_aggr(out=mv, in_=st6)
                    sd_ = spool.tile([128, 1], FP, tag="sd")
                    nc.scalar.activation(out=sd_, in_=mv[:, 1:2], func=AF.Sqrt, bias=eps)
                    rr = spool.tile([128, 1], FP, tag="rr")
                    nc.vector.reciprocal(out=rr, in_=sd_)
                    xn = apool.tile([128, 512], FR, tag="xn", bufs=8)
                    nc.vector.tensor_scalar(
                        out=xn, in0=z, scalar1=mv[:, 0:1], scalar2=rr,
                        op0=mybir.AluOpType.subtract, op1=mybir.AluOpType.mult,
                    )
                    if g_ap is not None:
                        gt_ = bcast_tile(g_ap, 512)
                        nc.vector.tensor_mul(xn, xn, gt_)
                    if b_ap is not None:
                        bt_ = bcast_tile(b_ap, 512)
                        nc.vector.tensor_add(xn, xn, bt_)
                    new_sd.append(xn)
                return new_sd, transpose_sd(new_sd, xt_tag)

            def residual(x_t, ps, badd_ap=None):
                """z = x_t + ps (+ badd broadcast); returns SBUF bf16 tile."""
                z = apool.tile([128, 512], FR, tag="z", bufs=4)
                nc.vector.tensor_add(z, x_t, ps)
                if badd_ap is not None:
                    bt = bcast_tile(badd_ap, 512)
                    nc.vector.tensor_add(z, z, bt)
                return z

            # ---- multi-head attention, transposed-score layout --------------
            mha_ctr = [0]

            def mha(x_sd, xq_ds, xkv_ds, w_ap, causal, badd_ap=None):
                """Fast path (no padding masks, no attn biases).
                Returns 4 SBUF z tiles (attn output + residual)."""
                midx = mha_ctr[0]; mha_ctr[0] += 1
                act_warm(AF.Exp)
                wq = [wtile(w_ap[0, ts(c, 128), :]) for c in range(DC)]
                wk = [wtile(w_ap[1, ts(c, 128), :]) for c in range(DC)]
                wv = [wtile(w_ap[2, ts(c, 128), :]) for c in range(DC)]
                wo = [wtile(w_ap[3, ds(64 * h, 64), :]) for h in range(H)]

                QT, KT = [], []
                for g in range(G):
                    psq = pspool.tile([128, 512], FP, tag="mm")
                    for c in range(DC):
                        nc.tensor.matmul(
                            out=psq, lhsT=wq[c][:, ts(g, 128)], rhs=xq_ds[c],
                            start=(c == 0), stop=(c == DC - 1),
                        )
                    qt = apool.tile([128, 512], FR, tag="qt", bufs=5)
                    nc.vector.tensor_copy(out=qt, in_=psq)
                    QT.append(qt)
                    psk = pspool.tile([128, 512], FP, tag="mm")
                    for c in range(DC):
                        nc.tensor.matmul(
                            out=psk, lhsT=wk[c][:, ts(g, 128)], rhs=xkv_ds[c],
                            start=(c == 0), stop=(c == DC - 1),
                        )
                    kt = apool.tile([128, 512], FR, tag="kt", bufs=5)
                    nc.vector.tensor_copy(out=kt, in_=psk)
                    KT.append(kt)

                # V with a ones column appended per head: [128 tok, 8*65]
                VO = []
                for t in range(TT):
                    psv = pspool.tile([128, 512], FP, tag="mm")
                    for c in range(DC):
                        nc.tensor.matmul(
                            out=psv, lhsT=xkv_ds[c][:, ts(t, 128)], rhs=wv[c],
                            start=(c == 0), stop=(c == DC - 1),
                        )
                    vvo = apool.tile([128, 8 * 65], FR, tag="vvo", bufs=5)
                    va = vvo[:, :]
                    nc.vector.memset(
                        bass.AP(tensor=va.tensor, offset=va.offset + 64,
                                ap=[va.ap[0], [65, 8], [1, 1]]), 1.0)
                    nc.vector.tensor_copy(
                        out=_view3(va, 8, 65, 64),
                        in_=_view3(psv[:, :], 8, 64, 64),
                    )
                    VO.append(vvo)
                if debug_stage == f"mha{midx}.qkv":
                    dump_and_stop(QT + KT)
                if debug_stage == f"mha{midx}.v":
                    dump_and_stop(VO)

                ctxT = []
                for g in range(G):
                    pscs = []
                    for lh in range(2):
                        h = 2 * g + lh
                        qsl = slice(64 * lh, 64 * lh + 64)
                        ES = []
                        for kt in range(TT):
                            q0 = 128 * kt if causal else 0
                            W = 512 - q0
                            scp = scpool.tile([128, 512], FP, tag="scp")
                            nc.tensor.matmul(
                                out=scp[:, ds(q0, W)],
                                lhsT=KT[g][qsl, ts(kt, 128)],
                                rhs=QT[g][qsl, ds(q0, W)],
                                start=True, stop=True,
                            )
                            if causal:
                                # add tril(-1e9, -1) to the diagonal block on
                                # DVE (frees a PE matmul+ldweights per k-tile)
                                nc.vector.tensor_add(
                                    scp[:, ds(q0, 128)], scp[:, ds(q0, 128)], tri)
                            es = apool.tile([128, 512], FR, tag="exp", bufs=8)
                            nc.scalar.activation(out=es[:, ds(q0, W)],
                                                 in_=scp[:, ds(q0, W)], func=AF.Exp)
                            ES.append(es)
                        psc = pcpool.tile([128, 512], FP, tag="psc")
                        for kt in range(TT):
                            q0 = 128 * kt if causal else 0
                            W = 512 - q0
                            nc.tensor.matmul(
                                out=psc[:65, ds(q0, W)],
                                lhsT=VO[kt][:, ds(65 * h, 65)],
                                rhs=ES[kt][:, ds(q0, W)],
                                start=(kt == 0), stop=(kt == TT - 1),
                            )
                        # softmax denominator: reciprocal of the ones-row, then
                        # partition-broadcast on the (idle) gpsimd engine
                        rec = spool.tile([1, 512], FR, tag="rec", bufs=4)
                        with nc.allow_low_precision(reason="bf16 softmax recip"):
                            nc.vector.reciprocal(out=rec, in_=psc[64:65, :])
                        rbs = apool.tile([64, 512], FR, tag="rbs", bufs=4)
                        nc.gpsimd.partition_broadcast(rbs[:, :], rec[:, :])
                        ct = apool.tile([64, 512], FR, tag="ctxh", bufs=10)
                        nc.vector.tensor_mul(ct, psc[:64, :], rbs)
                        ctxT.append(ct)

                if debug_stage == f"mha{midx}.ctx":
                    dump_and_stop(ctxT)
                z_sd = []
                for t in range(TT):
                    pso = pspool.tile([128, 512], FP, tag="mm")
                    for h in range(H):
                        nc.tensor.matmul(
                            out=pso, lhsT=ctxT[h][:, ts(t, 128)],
                            rhs=wo[h][:64, :],
                            start=(h == 0), stop=(h == H - 1),
                        )
                    z_sd.append(residual(x_sd[t], pso, badd_ap))
                if debug_stage == f"mha{midx}.out":
                    dump_and_stop(z_sd)
                return z_sd

            # ---- masked multi-head attention (general path) -----------------
            def mha_masked(x_sd, xq_ds, xkv_ds, w_ap, b_ap, causal, kmask,
                           badd_ap=None):
                midx = mha_ctr[0]; mha_ctr[0] += 1
                act_warm(AF.Exp)
                wq = [wtile(w_ap[0, ts(c, 128), :]) for c in range(DC)]
                wk = [wtile(w_ap[1, ts(c, 128), :]) for c in range(DC)]
                wv = [wtile(w_ap[2, ts(c, 128), :]) for c in range(DC)]
                wo = [wtile(w_ap[3, ds(64 * h, 64), :]) for h in range(H)]

                QT, KT = [], []
                for g in range(G):
                    psq = pspool.tile([128, 512], FP, tag="mm")
                    for c in range(DC):
                        nc.tensor.matmul(
                            out=psq, lhsT=wq[c][:, ts(g, 128)], rhs=xq_ds[c],
                            start=(c == 0), stop=(c == DC - 1),
                        )
                    qt = apool.tile([128, 512], FR, tag="qt", bufs=5)
                    if b_ap is not None:
                        nc.scalar.activation(out=qt, in_=psq, func=AF.Identity,
                                             bias=bias_col(b_ap[0, ts(g, 128)]))
                    else:
                        nc.scalar.copy(out=qt, in_=psq)
                    QT.append(qt)
                for g in range(G):
                    psk = pspool.tile([128, 512], FP, tag="mm")
                    for c in range(DC):
                        nc.tensor.matmul(
                            out=psk, lhsT=wk[c][:, ts(g, 128)], rhs=xkv_ds[c],
                            start=(c == 0), stop=(c == DC - 1),
                        )
                    kt = apool.tile([128, 512], FR, tag="kt", bufs=5)
                    if b_ap is not None:
                        nc.scalar.activation(out=kt, in_=psk, func=AF.Identity,
                                             bias=bias_col(b_ap[1, ts(g, 128)]))
                    else:
                        nc.scalar.copy(out=kt, in_=psk)
                    KT.append(kt)
                VV = []
                for t in range(TT):
                    psv = pspool.tile([128, 512], FP, tag="mm")
                    for c in range(DC):
                        nc.tensor.matmul(
                            out=psv, lhsT=xkv_ds[c][:, ts(t, 128)], rhs=wv[c],
                            start=(c == 0), stop=(c == DC - 1),
                        )
                    vv = apool.tile([128, 512], FR, tag="vvo", bufs=5)
                    if b_ap is not None:
                        bt = bcast_tile(b_ap[2], 512)
                        nc.vector.tensor_add(vv, psv, bt)
                    else:
                        nc.scalar.copy(out=vv, in_=psv)
                    VV.append(vv)

                ctxT = []
                for g in range(G):
                    for lh in range(2):
                        h = 2 * g + lh
                        psc = pcpool.tile([128, 512], FP, tag="psc")
                        qsl = slice(64 * lh, 64 * lh + 64)
                        att = []
                        for t in range(TT):
                            nch = TT if not causal else (t + 1)
                            W = 128 * nch
                            scp = scpool.tile([128, 512], FP, tag="scp")
                            nc.tensor.matmul(
                                out=scp[:, :W],
                                lhsT=QT[g][qsl, ts(t, 128)],
                                rhs=KT[g][qsl, :W],
                                start=True, stop=not causal,
                            )
                            if causal:
                                nc.tensor.matmul(
                                    out=scp[:, ds(128 * t, 128)],
                                    lhsT=tri, rhs=ident,
                                    start=False, stop=True,
                                )
                            if kmask is not None:
                                nc.vector.tensor_add(scp[:, :W], scp[:, :W], kmask[:, :W])
                            if causal and flags.get("tgt_mask"):
                                nc.vector.tensor_scalar_mul(scp[:, :W], scp[:, :W],
                                                            rmt[:, t : t + 1])
                                nc.vector.tensor_add(scp[:, :W], scp[:, :W], cmt[t][:, :W])
                            et = apool.tile([128, 512], FR, tag="exp", bufs=8)
                            ssum = spool.tile([128, 1], FP, tag="ssum")
                            nc.scalar.activation(out=et[:, :W], in_=scp[:, :W],
                                                 func=AF.Exp, accum_out=ssum)
                            rr = spool.tile([128, 1], FP, tag="srr")
                            nc.vector.reciprocal(out=rr, in_=ssum)
                            nc.vector.tensor_scalar_mul(et[:, :W], et[:, :W], rr)
                            att.append(et)
                        for c in range(TT):
                            t0 = c if causal else 0
                            wq_ = 512 - 128 * t0
                            trp = scpool.tile([128, 512], FR, tag="scp")
                            for t in range(t0, TT):
                                nc.tensor.transpose(
                                    out=trp[:, ds(128 * t, 128)],
                                    in_=att[t][:, ds(128 * c, 128)],
                                    identity=ident,
                                )
                            aT = apool.tile([128, 512], FR, tag="rbs", bufs=4)
                            nc.any.tensor_copy(out=aT[:, ds(128 * t0, wq_)],
                                               in_=trp[:, ds(128 * t0, wq_)])
                            nc.tensor.matmul(
                                out=psc[:64, ds(128 * t0, wq_)],
                                lhsT=VV[c][:, ds(64 * h, 64)],
                                rhs=aT[:, ds(128 * t0, wq_)],
                                start=(c == 0), stop=(c == TT - 1),
                            )
                        ct = apool.tile([64, 512], FR, tag="ctxh", bufs=10)
                        if b_ap is not None:
                            nc.scalar.activation(out=ct, in_=psc[:64, :],
                                                 func=AF.Identity,
                                                 bias=bias_col(b_ap[2, ds(64 * h, 64)]))
                        else:
                            nc.scalar.copy(out=ct, in_=psc[:64, :])
                        ctxT.append(ct)

                z_sd = []
                for t in range(TT):
                    pso = pspool.tile([128, 512], FP, tag="mm")
                    for h in range(H):
                        nc.tensor.matmul(
                            out=pso, lhsT=ctxT[h][:, ts(t, 128)],
                            rhs=wo[h][:64, :],
                            start=(h == 0), stop=(h == H - 1),
                        )
                    z_sd.append(residual(x_sd[t], pso, badd_ap))
                return z_sd

            def mha_any(x_sd, xq_ds, xkv_ds, w_ap, b_ap, causal, kmask,
                        badd_ap=None):
                use_masked = (b_ap is not None or kmask is not None
                              or (causal and flags.get("tgt_mask")))
                if use_masked:
                    return mha_masked(x_sd, xq_ds, xkv_ds, w_ap, b_ap, causal,
                                      kmask, badd_ap)
                return mha(x_sd, xq_ds, xkv_ds, w_ap, causal, badd_ap)

            # ---- FFN --------------------------------------------------------
            def ffn(x_sd, x_ds, w1_ap, w2_ap, b1_ap, b2_ap):
                w1 = [[wtile(w1_ap[ts(c, 128), ts(g2, 512)]) for g2 in range(4)]
                      for c in range(DC)]
                w2 = [wtile(w2_ap[ts(dt, 128), :]) for dt in range(FTL)]
                hT = []
                for dt in range(FTL):
                    g2, r = dt // 4, dt % 4
                    psh = pspool.tile([128, 512], FP, tag="mm")
                    for c in range(DC):
                        nc.tensor.matmul(
                            out=psh, lhsT=w1[c][g2][:, ds(128 * r, 128)], rhs=x_ds[c],
                            start=(c == 0), stop=(c == DC - 1),
                        )
                    ht = hpool.tile([128, 512], FR, tag="hT")
                    if b1_ap is not None:
                        nc.scalar.activation(out=ht, in_=psh, func=AF.Relu,
                                             bias=bias_col(b1_ap[ts(dt, 128)]))
                    else:
                        nc.scalar.activation(out=ht, in_=psh, func=AF.Relu)
                    hT.append(ht)
                z_sd = []
                for t in range(TT):
                    psf = pspool.tile([128, 512], FP, tag="mm")
                    for dt in range(FTL):
                        nc.tensor.matmul(
                            out=psf, lhsT=hT[dt][:, ts(t, 128)], rhs=w2[dt],
                            start=(dt == 0), stop=(dt == FTL - 1),
                        )
                    z_sd.append(residual(x_sd[t], psf, b2_ap))
                return z_sd

            # ================= encoder =================
            x_sd = []
            for t in range(TT):
                xt_ = apool.tile([128, 512], FR, tag="xn", bufs=8)
                nc.sync.dma_start(out=xt_, in_=x0e[ts(t, 128), :])
                x_sd.append(xt_)
            x_ds = transpose_sd(x_sd, "xT")
            if debug_stage == "x0":
                dump_and_stop(x_sd + x_ds)

            for i in range(LE):
                ab = be_attn[i] if flags.get("attn_bias") else None
                bo = be_attn[i, 3] if flags.get("ffn_bias") else None
                z = mha_any(x_sd, x_ds, x_ds, we_attn[i], ab, False, bcast, bo)
                lg = eln_g[i, 0] if flags.get("ln_affine") else None
                lb = eln_b[i, 0] if flags.get("ln_affine") else None
                x_sd, x_ds = ln_block(z, "xT", lg, lb)
                if debug_stage == f"enc{i}.ln1":
                    dump_and_stop(x_sd + x_ds)
                b1 = be_f1[i] if flags.get("ffn_bias") else None
                b2 = be_f2[i] if flags.get("ffn_bias") else None
                z = ffn(x_sd, x_ds, we_f1[i], we_f2[i], b1, b2)
                lg = eln_g[i, 1] if flags.get("ln_affine") else None
                lb = eln_b[i, 1] if flags.get("ln_affine") else None
                x_sd, x_ds = ln_block(z, "xT", lg, lb)
                if debug_stage == f"enc{i}":
                    dump_and_stop(x_sd + x_ds)

            # persist encoder output (transposed) for cross attention
            encT = []
            for c in range(DC):
                e = apool.tile([128, 512], FR, tag="encT", bufs=4)
                nc.any.tensor_copy(out=e, in_=x_ds[c])
                encT.append(e)

            # ================= decoder =================
            y_sd = []
            for t in range(TT):
                yt_ = apool.tile([128, 512], FR, tag="xn", bufs=8)
                nc.sync.dma_start(out=yt_, in_=x0d[ts(t, 128), :])
                y_sd.append(yt_)
            y_ds = transpose_sd(y_sd, "xT")

            for i in range(LD):
                ab = bd_sa[i] if flags.get("attn_bias") else None
                bo = bd_sa[i, 3] if flags.get("ffn_bias") else None
                z = mha_any(y_sd, y_ds, y_ds, wd_sa[i], ab, True, None, bo)
                lg = dln_g[i, 0] if flags.get("ln_affine") else None
                lb = dln_b[i, 0] if flags.get("ln_affine") else None
                y_sd, y_ds = ln_block(z, "xT", lg, lb)

                ab = bd_ca[i] if flags.get("attn_bias") else None
                bo = bd_ca[i, 3] if flags.get("ffn_bias") else None
                z = mha_any(y_sd, y_ds, encT, wd_ca[i], ab, False, bcast, bo)
                lg = dln_g[i, 1] if flags.get("ln_affine") else None
                lb = dln_b[i, 1] if flags.get("ln_affine") else None
                y_sd, y_ds = ln_block(z, "xT", lg, lb)

                b1 = bd_f1[i] if flags.get("ffn_bias") else None
                b2 = bd_f2[i] if flags.get("ffn_bias") else None
                z = ffn(y_sd, y_ds, wd_f1[i], wd_f2[i], b1, b2)
                lg = dln_g[i, 2] if flags.get("ln_affine") else None
                lb = dln_b[i, 2] if flags.get("ln_affine") else None
                y_sd, y_ds = ln_block(z, "xT", lg, lb)
                if debug_stage == f"dec{i}":
                    dump_and_stop(y_sd + y_ds)

            # ================= output projection =================
            nvc = (VH + 511) // 512
            for vc in range(nvc):
                w_ = min(512, VH - 512 * vc)
                wo_c = []
                for c in range(DC):
                    wt_ = wpool.tile([128, 512], FR, tag="w")
                    nc.sync.dma_start(out=wt_[:, :w_],
                                      in_=wout[ts(c, 128), ds(512 * vc, w_)])
                    wo_c.append(wt_)
                for t in range(TT):
                    pso = pspool.tile([128, 512], FP, tag="mm")
                    for c in range(DC):
                        nc.tensor.matmul(
                            out=pso[:, :w_], lhsT=y_ds[c][:, ts(t, 128)],
                            rhs=wo_c[c][:, :w_],
                            start=(c == 0), stop=(c == DC - 1),
                        )
                    sb = apool.tile([128, 512], FP, tag="ob", bufs=4)
                    if flags.get("out_bias"):
                        ob = bcast_tile(bout[ds(512 * vc, w_)], w_)
                        nc.vector.tensor_add(sb[:, :w_], pso[:, :w_], ob)
                    elif (vc * TT + t) % 2 == 0:
                        # alternate PSUM drains across ACT/DVE so the two
                        # mm slots recycle twice as fast in the logits tail
                        nc.scalar.copy(out=sb[:, :w_], in_=pso[:, :w_])
                    else:
                        nc.vector.tensor_copy(out=sb[:, :w_], in_=pso[:, :w_])
                    nc.sync.dma_start(out=logits[ts(t, 128), ds(512 * vc, w_)],
                                      in_=sb[:, :w_])
      except _StopTrace:
          pass

    nc.finalize()
    return nc


def _build_fast(debug_stage=None):
    """v2 builder: no masks/biases/affine-LN.  Single transposed activation
    layout xT:[D,S] (4 chunks of [128,512]).  No PE transposes anywhere:
      - LN stats via ones-column matmuls into one PSUM accumulation group
        ([2,512]: row0 = -mean, row1 = E[z^2]); scale/shift row broadcast via
        gpsimd; apply = 2 DVE passes per chunk.
      - attention out-proj and FFN w2 produce transposed outputs directly
        (lhsT = weight chunk, rhs = ctx/h in dim-partition layout), K=128.
      - decoder-causal scores packed two heads per PE pass via row-group
        tile_position (lhsT base partitions 0/64).
      - logits tail reuses the y stationary across 8 vocab chunks resident
        in all 8 PSUM banks.
    """
    nc = bacc.Bacc(None)

    x0eT = nc.declare_dram_parameter("x0eT", [D, S], FR, isOutput=False)
    x0dT = nc.declare_dram_parameter("x0dT", [D, S], FR, isOutput=False)
    we_attn = nc.declare_dram_parameter("we_attn", [LE, 4, D, D], FR, isOutput=False)
    we_f1 = nc.declare_dram_parameter("we_f1", [LE, D, DFF], FR, isOutput=False)
    we_f2 = nc.declare_dram_parameter("we_f2", [LE, DFF, D], FR, isOutput=False)
    wd_sa = nc.declare_dram_parameter("wd_sa", [LD, 4, D, D], FR, isOutput=False)
    wd_ca = nc.declare_dram_parameter("wd_ca", [LD, 4, D, D], FR, isOutput=False)
    wd_f1 = nc.declare_dram_parameter("wd_f1", [LD, D, DFF], FR, isOutput=False)
    wd_f2 = nc.declare_dram_parameter("wd_f2", [LD, DFF, D], FR, isOutput=False)
    wout = nc.declare_dram_parameter("wout", [D, VH], FR, isOutput=False)
    ctri = nc.declare_dram_parameter("ctri", [128, 128], FR, isOutput=False)
    cstat = nc.declare_dram_parameter("cstat", [128, 256], FR, isOutput=False)
    logits = nc.declare_dram_parameter("logits", [S, VH], FR, isOutput=True)
    dbg = None
    if debug_stage is not None:
        dbg = nc.declare_dram_parameter("dbg", [8, 128, 512], FP, isOutput=True)

    from contextlib import ExitStack

    with tile.TileContext(nc) as tc, ExitStack() as stk:
      try:
        wpool = stk.enter_context(tc.tile_pool(name="w", bufs=72))
        apool = stk.enter_context(tc.tile_pool(name="acts", bufs=2))
        hpool = stk.enter_context(tc.tile_pool(name="h", bufs=16))
        cpool = stk.enter_context(tc.tile_pool(name="consts", bufs=1))
        spool = stk.enter_context(tc.tile_pool(name="small", bufs=12))
        pspool = stk.enter_context(tc.tile_pool(name="ps", bufs=2, space="PSUM"))
        scpool = stk.enter_context(tc.tile_pool(name="sc", bufs=2, space="PSUM"))
        pcpool = stk.enter_context(tc.tile_pool(name="pc", bufs=2, space="PSUM"))
        stpool = stk.enter_context(tc.tile_pool(name="st", bufs=2, space="PSUM"))
        if True:
            # ---- constants --------------------------------------------------
            tri = cpool.tile([128, 128], FR)
            nc.sync.dma_start(out=tri, in_=ctri[:, :])
            statw = cpool.tile([128, 256], FR)
            nc.sync.dma_start(out=statw, in_=cstat[:, :])
            eps = cpool.tile([128, 1], FP)
            nc.vector.memset(eps, 1e-5)

            junk = cpool.tile([128, 1], FP, tag="junk")
            nc.vector.memset(junk, 1.0)

            def act_warm(func):
                # dummy activation to pull the ACT function table in while
                # the PE is busy, so the real op doesn't pay the ~1.3us
                # table reload on the LN critical path
                j = spool.tile([128, 1], FP, tag="jk", bufs=2)
                nc.scalar.activation(out=j, in_=junk, func=func)

            def wtile(dram_ap, tag="w"):
                t_ = wpool.tile([128, 512], FR, tag=tag)
                nc.sync.dma_start(out=t_, in_=dram_ap)
                return t_

            def dump_and_stop(tiles):
                for j, t_ in enumerate(tiles[:8]):
                    stt = apool.tile([128, 512], FP, tag="ob", bufs=4)
                    pp, ff = t_.shape[-2], t_.shape[-1]
                    nc.any.tensor_copy(out=stt[:pp, :ff], in_=t_)
                    nc.sync.dma_start(out=dbg[j, :pp, :ff], in_=stt[:pp, :ff])
                raise _StopTrace

            # ---- layernorm over the partition (D) axis ----------------------
            # z chunks are [128,512] bf16 (dims x tokens).  Stats:
            #   pstat[0,:] = sum_c (-1/512) * z_c   (= -mean)
            #   pstat[1,:] = sum_c (1/512) * z_c^2  (= E[z^2])
            # one PSUM accumulation group; two stationaries (cstat cols).
            def ln_T(z_ds, out_tag, out_bufs=8):
                # stats matmuls use a [128,128] constant stationary (+-1/512),
                # so every output partition carries the same row: the stats
                # arrive pre-broadcast and no gpsimd partition_broadcast is
                # needed anywhere in the chain.
                act_warm(AF.Abs_reciprocal_sqrt)
                pstatA = stpool.tile([128, 512], FP, tag="pstat")  # -mean
                pstatB = stpool.tile([128, 512], FP, tag="pstat")  # E[z^2]
                zsq = []
                for c in range(DC):
                    zq = apool.tile([128, 512], FR, tag="zsq", bufs=4)
                    nc.vector.tensor_mul(zq, z_ds[c], z_ds[c])
                    zsq.append(zq)
                for c in range(DC):
                    nc.tensor.matmul(
                        out=pstatA, lhsT=statw[:, 0:128], rhs=z_ds[c],
                        start=(c == 0), stop=(c == DC - 1),
                    )
                for c in range(DC):
                    nc.tensor.matmul(
                        out=pstatB, lhsT=statw[:, 128:256], rhs=zsq[c],
                        start=(c == 0), stop=(c == DC - 1),
                    )
                # centered z (zc = z - mu) runs off pstatA while the PE does
                # the sq stats matmuls; only the final scale multiply waits on
                # the rsqrt.
                stage = apool.tile([128, 512], FP, tag="stg", bufs=2)
                nc.vector.tensor_copy(out=stage, in_=pstatA)
                musq = apool.tile([128, 512], FP, tag="musq", bufs=2)
                nc.vector.tensor_mul(musq, stage, stage)
                var = apool.tile([128, 512], FP, tag="var", bufs=2)
                nc.vector.tensor_sub(var, pstatB, musq)
                bcA = apool.tile([128, 512], FR, tag="lnbcA", bufs=2)
                nc.scalar.activation(out=bcA, in_=var,
                                     func=AF.Abs_reciprocal_sqrt, bias=eps)
                bcB = apool.tile([128, 512], FR, tag="lnbcB", bufs=2)
                with nc.allow_low_precision(reason="bf16 LN shift"):
                    nc.vector.tensor_mul(bcB, stage, bcA)
                xn_ds = []
                for c in range(DC):
                    xn = apool.tile([128, 512], FR, tag=out_tag, bufs=out_bufs)
                    nc.vector.tensor_mul(xn, z_ds[c], bcA)
                    nc.vector.tensor_add(xn, xn, bcB)
                    xn_ds.append(xn)
                return xn_ds

            # ---- multi-head attention (transposed everything) ---------------
            # persistent V ring: per head 64 V-dim cols + 64 static ones cols
            # ([128, 8*128]); the ones make the ctx matmul emit the softmax
            # denominator pre-broadcast on psum partitions 64-127.
            NVR = 6
            vvo_ring = [apool.tile([128, 1024], FR, tag="vvo", bufs=NVR,
                                   name=f"vvo{i}") for i in range(NVR)]
            for i in range(NVR):
                nc.vector.memset(
                    _view3(vvo_ring[i][:, :], 8, 128, 64, inner_off=64), 1.0)
            vv_ctr = [0]
            mha_ctr = [0]

            def mha_T(xqT, xkvT, w_ap, causal):
                """Returns 4 z chunks [128,512] bf16 in [D,S] layout
                (attention output + residual xqT)."""
                midx = mha_ctr[0]; mha_ctr[0] += 1
                act_warm(AF.Exp)
                wq = [wtile(w_ap[0, ts(c, 128), :]) for c in range(DC)]
                wk = [wtile(w_ap[1, ts(c, 128), :]) for c in range(DC)]
                wv = [wtile(w_ap[2, ts(c, 128), :]) for c in range(DC)]
                wo = [wtile(w_ap[3, ts(p, 128), :]) for p in range(G)]

                QT, KT = [], []
                for g in range(G):
                    psq = pspool.tile([128, 512], FP, tag="mm")
                    for c in range(DC):
                        nc.tensor.matmul(
                            out=psq, lhsT=wq[c][:, ts(g, 128)], rhs=xqT[c],
                            start=(c == 0), stop=(c == DC - 1),
                        )
                    qt = apool.tile([128, 512], FR, tag="qt", bufs=5)
                    nc.vector.tensor_copy(out=qt, in_=psq)
                    QT.append(qt)
                    psk = pspool.tile([128, 512], FP, tag="mm")
                    for c in range(DC):
                        nc.tensor.matmul(
                            out=psk, lhsT=wk[c][:, ts(g, 128)], rhs=xkvT[c],
                            start=(c == 0), stop=(c == DC - 1),
                        )
                    kt = apool.tile([128, 512], FR, tag="kt", bufs=5)
                    nc.vector.tensor_copy(out=kt, in_=psk)
                    KT.append(kt)

                # V ring tiles: write the 8x64 V views; ones cols persist
                VO = []
                for t in range(TT):
                    psv = pspool.tile([128, 512], FP, tag="mm")
                    for c in range(DC):
                        nc.tensor.matmul(
                            out=psv, lhsT=xkvT[c][:, ts(t, 128)], rhs=wv[c],
                            start=(c == 0), stop=(c == DC - 1),
                        )
                    vvo = vvo_ring[vv_ctr[0] % NVR]
                    vv_ctr[0] += 1
                    nc.vector.tensor_copy(
                        out=_view3(vvo[:, :], 8, 128, 64),
                        in_=_view3(psv[:, :], 8, 64, 64),
                    )
                    VO.append(vvo)
                if debug_stage == f"mha{midx}.qkv":
                    dump_and_stop(QT + KT)
                if debug_stage == f"mha{midx}.v":
                    dump_and_stop(VO)

                ctxP = [apool.tile([128, 512], FR, tag="ctxP", bufs=6,
                                   name=f"ctxP{p}") for p in range(G)]
                for g in range(G):
                    # scores for the two heads of this pair run concurrently
                    # on PE row-groups 0-1 / 2-3 (lhsT base partition 0 / 64).
                    ES = [[], []]
                    for kt in range(TT):
                        q0 = 128 * kt if causal else 0
                        W = 512 - q0
                        scps = []
                        for lh in range(2):
                            qsl = slice(64 * lh, 64 * lh + 64)
                            scp = scpool.tile([128, 512], FP, tag="scp")
                            nc.tensor.matmul(
                                out=scp[:, ds(q0, W)],
                                lhsT=KT[g][qsl, ts(kt, 128)],
                                rhs=QT[g][qsl, ds(q0, W)],
                                start=True, stop=True,
                            )
                            scps.append(scp)
                        for lh in range(2):
                            scp = scps[lh]
                            if causal:
                                nc.vector.tensor_add(
                                    scp[:, ds(q0, 128)], scp[:, ds(q0, 128)], tri)
                            es = apool.tile([128, 512], FR, tag="exp", bufs=10)
                            nc.scalar.activation(out=es[:, ds(q0, W)],
                                                 in_=scp[:, ds(q0, W)], func=AF.Exp)
                            ES[lh].append(es)
                    for lh in range(2):
                        h = 2 * g + lh
                        psc = pcpool.tile([128, 512], FP, tag="psc")
                        for kt in range(TT):
                            q0 = 128 * kt if causal else 0
                            W = 512 - q0
                            nc.tensor.matmul(
                                out=psc[:, ds(q0, W)],
                                lhsT=VO[kt][:, ds(128 * h, 128)],
                                rhs=ES[lh][kt][:, ds(q0, W)],
                                start=(kt == 0), stop=(kt == TT - 1),
                            )
                        den = spool.tile([64, 512], FP, tag="den", bufs=4)
                        nc.vector.tensor_copy(out=den, in_=psc[64:128, :])
                        rec = spool.tile([64, 512], FP, tag="rec", bufs=4)
                        nc.vector.reciprocal_approx_fast(out=rec, in_=den)
                        nc.vector.tensor_mul(
                            ctxP[g][64 * lh: 64 * lh + 64, :],
                            psc[:64, :], rec)

                if debug_stage == f"mha{midx}.ctx":
                    dump_and_stop(ctxP)
                z_ds = []
                for c in range(DC):
                    pso = pspool.tile([128, 512], FP, tag="mm")
                    for p in range(G):
                        nc.tensor.matmul(
                            out=pso, lhsT=wo[p][:, ts(c, 128)], rhs=ctxP[p],
                            start=(p == 0), stop=(p == G - 1),
                        )
                    z = apool.tile([128, 512], FR, tag="z", bufs=4)
                    nc.vector.tensor_add(z, xqT[c], pso)
                    z_ds.append(z)
                if debug_stage == f"mha{midx}.out":
                    dump_and_stop(z_ds)
                return z_ds

            # ---- FFN (transposed output) ------------------------------------
            def ffn_T(xT, w1_ap, w2_ap):
                act_warm(AF.Relu)
                w1 = [[wtile(w1_ap[ts(c, 128), ts(g2, 512)]) for g2 in range(4)]
                      for c in range(DC)]
                w2 = [wtile(w2_ap[ts(dt, 128), :]) for dt in range(FTL)]
                hT = []
                for dt in range(FTL):
                    g2, r = dt // 4, dt % 4
                    psh = pspool.tile([128, 512], FP, tag="mm")
                    for c in range(DC):
                        nc.tensor.matmul(
                            out=psh, lhsT=w1[c][g2][:, ds(128 * r, 128)], rhs=xT[c],
                            start=(c == 0), stop=(c == DC - 1),
                        )
                    ht = hpool.tile([128, 512], FR, tag="hT")
                    nc.scalar.activation(out=ht, in_=psh, func=AF.Relu)
                    hT.append(ht)
                z_ds = []
                for c in range(DC):
                    psf = pspool.tile([128, 512], FP, tag="mm")
                    for dt in range(FTL):
                        nc.tensor.matmul(
                            out=psf, lhsT=w2[dt][:, ts(c, 128)], rhs=hT[dt],
                            start=(dt == 0), stop=(dt == FTL - 1),
                        )
                    z = apool.tile([128, 512], FR, tag="z", bufs=4)
                    nc.vector.tensor_add(z, xT[c], psf)
                    z_ds.append(z)
                return z_ds

            # ================= encoder =================
            x_ds = []
            for c in range(DC):
                xt_ = apool.tile([128, 512], FR, tag="xn", bufs=8)
                nc.sync.dma_start(out=xt_, in_=x0eT[ts(c, 128), :])
                x_ds.append(xt_)
            if debug_stage == "x0":
                dump_and_stop(x_ds)

            for i in range(LE):
                z = mha_T(x_ds, x_ds, we_attn[i], False)
                x_ds = ln_T(z, "xn")
                if debug_stage == f"enc{i}.ln1":
                    dump_and_stop(x_ds)
                z = ffn_T(x_ds, we_f1[i], we_f2[i])
                out_tag = "encT" if i == LE - 1 else "xn"
                x_ds = ln_T(z, out_tag, out_bufs=4 if i == LE - 1 else 8)
                if debug_stage == f"enc{i}":
                    dump_and_stop(x_ds)
            encT = x_ds

            # ================= decoder =================
            y_ds = []
            for c in range(DC):
                yt_ = apool.tile([128, 512], FR, tag="xn", bufs=8)
                nc.sync.dma_start(out=yt_, in_=x0dT[ts(c, 128), :])
                y_ds.append(yt_)

            for i in range(LD):
                z = mha_T(y_ds, y_ds, wd_sa[i], True)
                y_ds = ln_T(z, "xn")
                z = mha_T(y_ds, encT, wd_ca[i], False)
                y_ds = ln_T(z, "xn")
                z = ffn_T(y_ds, wd_f1[i], wd_f2[i])
                out_tag = "y" if i == LD - 1 else "xn"
                y_ds = ln_T(z, out_tag, out_bufs=4 if i == LD - 1 else 8)
                if debug_stage == f"dec{i}":
                    dump_and_stop(y_ds)

            # ================= output projection =================
            # 32 vocab chunks of <=512, in groups of 8 (one PSUM bank each);
            # stationary y[c][:, t-block] is reused across the 8 chunks.
            nvc = (VH + 511) // 512
            psum_of = [pspool, pspool, scpool, scpool,
                       pcpool, pcpool, stpool, stpool]
            ptag = ["mm", "mm", "scp", "scp", "psc", "psc", "pstat", "pstat"]
            for vg in range(0, nvc, 8):
                grp = list(range(vg, min(vg + 8, nvc)))
                wt_g = {}
                for c in range(DC):
                    for j in grp:
                        w_ = min(512, VH - 512 * j)
                        wt_ = wpool.tile([128, 512], FR, tag="w")
                        nc.sync.dma_start(out=wt_[:, :w_],
                                          in_=wout[ts(c, 128), ds(512 * j, w_)])
                        wt_g[(c, j)] = wt_
                for t in range(TT):
                    pss = [psum_of[k].tile([128, 512], FP, tag=ptag[k],
                                           name=f"pl{k}") for k in range(len(grp))]
                    for c in range(DC):
                        for k, j in enumerate(grp):
                            w_ = min(512, VH - 512 * j)
                            nc.tensor.matmul(
                                out=pss[k][:, :w_],
                                lhsT=y_ds[c][:, ts(t, 128)],
                                rhs=wt_g[(c, j)][:, :w_],
                                start=(c == 0), stop=(c == DC - 1),
                            )
                    for k, j in enumerate(grp):
                        w_ = min(512, VH - 512 * j)
                        sb = apool.tile([128, 512], FR, tag="ob", bufs=8)
                        if k % 2 == 0:
                            nc.scalar.copy(out=sb[:, :w_], in_=pss[k][:, :w_])
                        else:
                            nc.vector.tensor_copy(out=sb[:, :w_], in_=pss[k][:, :w_])
                        nc.sync.dma_start(out=logits[ts(t, 128), ds(512 * j, w_)],
                                          in_=sb[:, :w_])
      except _StopTrace:
          pass

    nc.finalize()
    return nc


def _host_prep(inputs):
    """Host-side preparation: embeddings, weight folding, masks, flags."""
    src = np.asarray(inputs["src"])
    tgt = np.asarray(inputs["tgt"])
    f32 = lambda k: np.ascontiguousarray(np.asarray(inputs[k], dtype=np.float32))

    enc_emb, dec_emb, pe = f32("enc_emb"), f32("dec_emb"), f32("pe")
    sqd = np.float32(math.sqrt(D))
    x0e = enc_emb[src] * sqd + pe[None, :S]          # [B, S, D]
    x0d = dec_emb[tgt] * sqd + pe[None, :S]

    we_attn = f32("enc_attn_w").copy()
    wd_sa = f32("dec_sa_w").copy()
    wd_ca = f32("dec_ca_w").copy()
    scale = np.float32(1.0 / math.sqrt(DK))
    we_attn[:, 0] *= scale
    wd_sa[:, 0] *= scale
    wd_ca[:, 0] *= scale
    be_attn = f32("enc_attn_b").copy()
    bd_sa = f32("dec_sa_b").copy()
    bd_ca = f32("dec_ca_b").copy()
    be_attn[:, 0] *= scale
    bd_sa[:, 0] *= scale
    bd_ca[:, 0] *= scale

    flags = {
        "attn_bias": bool(np.any(be_attn[:, (0, 2)]) or np.any(bd_sa[:, (0, 2)])
                          or np.any(bd_ca[:, (0, 2)])),
        "ffn_bias": bool(np.any(f32("enc_ffn_b1")) or np.any(f32("enc_ffn_b2"))
                         or np.any(f32("dec_ffn_b1")) or np.any(f32("dec_ffn_b2"))
                         or np.any(be_attn[:, 3]) or np.any(bd_sa[:, 3])
                         or np.any(bd_ca[:, 3])),
        "ln_affine": bool(np.any(f32("enc_ln_g") != 1.0) or np.any(f32("enc_ln_b"))
                          or np.any(f32("dec_ln_g") != 1.0) or np.any(f32("dec_ln_b"))),
        "out_bias": bool(np.any(f32("out_b"))),
        "src_mask": bool((src == 0).any()),
        "tgt_mask": bool((tgt == 0).any()),
    }

    bf = lambda a: np.ascontiguousarray(np.asarray(a, np.float32).astype(BFNP))
    cident = np.eye(128, dtype=np.float32)
    ctri = np.tril(np.full((128, 128), NEG, dtype=np.float32), k=-1)

    common = {
        "we_attn": bf(we_attn),
        "we_f1": bf(f32("enc_ffn_w1")), "we_f2": bf(f32("enc_ffn_w2")),
        "wd_sa": bf(wd_sa), "wd_ca": bf(wd_ca),
        "wd_f1": bf(f32("dec_ffn_w1")), "wd_f2": bf(f32("dec_ffn_w2")),
        "cident": bf(cident), "ctri": bf(ctri),
        "cones": np.ones((1, 64), np.float32),
    }
    if flags["attn_bias"]:
        common.update(be_attn=np.ascontiguousarray(be_attn),
                      bd_sa=np.ascontiguousarray(bd_sa),
                      bd_ca=np.ascontiguousarray(bd_ca))
    if flags["ffn_bias"]:
        common.update(be_f1=f32("enc_ffn_b1"), be_f2=f32("enc_ffn_b2"),
                      bd_f1=f32("dec_ffn_b1"), bd_f2=f32("dec_ffn_b2"))
    if flags["ln_affine"]:
        common.update(eln_g=f32("enc_ln_g"), eln_b=f32("enc_ln_b"),
                      dln_g=f32("dec_ln_g"), dln_b=f32("dec_ln_b"))

    out_w = f32("out_w")
    out_b = f32("out_b")

    in_maps = []
    for core in range(NCORES):
        b, half = core // 2, core % 2
        m = dict(common)
        m["x0e"] = bf(x0e[b])
        m["x0d"] = bf(x0d[b])
        m["wout"] = bf(out_w[:, half * VH : (half + 1) * VH])
        if flags["out_bias"]:
            m["bout"] = np.ascontiguousarray(out_b[half * VH : (half + 1) * VH])
        if flags["src_mask"]:
            m["km_src"] = np.where(src[b] != 0, 0.0, NEG).astype(np.float32)
        if flags["tgt_mask"]:
            rm = (tgt[b] != 0).astype(np.float32)
            m["rm_tgt"] = rm
            cm = np.where(np.tril(np.ones((S, S), bool)), 0.0, NEG).astype(np.float32)
            cm = cm * rm[:, None]          # padded query rows -> all-zero scores
            m["cm_tgt"] = np.ascontiguousarray(cm)
        in_maps.append(m)

    return flags, in_maps


def _any_special(inputs):
    """True if any mask/bias/affine feature is active (v1 fallback needed)."""
    f32 = lambda k: np.asarray(inputs[k], dtype=np.float32)
    src = np.asarray(inputs["src"])
    tgt = np.asarray(inputs["tgt"])
    return bool(
        np.any(f32("enc_attn_b")) or np.any(f32("dec_sa_b"))
        or np.any(f32("dec_ca_b"))
        or np.any(f32("enc_ffn_b1")) or np.any(f32("enc_ffn_b2"))
        or np.any(f32("dec_ffn_b1")) or np.any(f32("dec_ffn_b2"))
        or np.any(f32("enc_ln_g") != 1.0) or np.any(f32("enc_ln_b"))
        or np.any(f32("dec_ln_g") != 1.0) or np.any(f32("dec_ln_b"))
        or np.any(f32("out_b"))
        or (src == 0).any() or (tgt == 0).any()
    )


def _host_prep_fast(inputs):
    src = np.asarray(inputs["src"])
    tgt = np.asarray(inputs["tgt"])
    f32 = lambda k: np.ascontiguousarray(np.asarray(inputs[k], dtype=np.float32))

    enc_emb, dec_emb, pe = f32("enc_emb"), f32("dec_emb"), f32("pe")
    sqd = np.float32(math.sqrt(D))
    x0e = enc_emb[src] * sqd + pe[None, :S]          # [B, S, D]
    x0d = dec_emb[tgt] * sqd + pe[None, :S]

    we_attn = f32("enc_attn_w").copy()
    wd_sa = f32("dec_sa_w").copy()
    wd_ca = f32("dec_ca_w").copy()
    scale = np.float32(1.0 / math.sqrt(DK))
    we_attn[:, 0] *= scale
    wd_sa[:, 0] *= scale
    wd_ca[:, 0] *= scale

    bf = lambda a: np.ascontiguousarray(np.asarray(a, np.float32).astype(BFNP))
    ctri = np.tril(np.full((128, 128), NEG, dtype=np.float32), k=-1)
    cstat = np.zeros((128, 256), np.float32)
    cstat[:, 0:128] = -1.0 / 512
    cstat[:, 128:256] = 1.0 / 512

    common = {
        "we_attn": bf(we_attn),
        "we_f1": bf(f32("enc_ffn_w1")), "we_f2": bf(f32("enc_ffn_w2")),
        "wd_sa": bf(wd_sa), "wd_ca": bf(wd_ca),
        "wd_f1": bf(f32("dec_ffn_w1")), "wd_f2": bf(f32("dec_ffn_w2")),
        "ctri": bf(ctri), "cstat": bf(cstat),
    }
    out_w = f32("out_w")
    in_maps = []
    for core in range(NCORES):
        b, half = core // 2, core % 2
        m = dict(common)
        m["x0eT"] = bf(np.ascontiguousarray(x0e[b].T))
        m["x0dT"] = bf(np.ascontiguousarray(x0d[b].T))
        m["wout"] = bf(out_w[:, half * VH: (half + 1) * VH])
        in_maps.append(m)
    return in_maps


def _run(inputs, trace=False, debug_stage=None, **kwargs):
    if not _any_special(inputs):
        in_maps = _host_prep_fast(inputs)
        nc = _build_fast(debug_stage)
        res = run_bass_kernel_spmd(nc, in_maps, list(range(NCORES)),
                                   trace=trace, **kwargs)
        out = np.zeros((4, S, V), dtype=np.float32)
        for core in range(NCORES):
            b, half = core // 2, core % 2
            out[b, :, half * VH: (half + 1) * VH] = np.asarray(
                res.results[core]["logits"], np.float32)
        return out, res
    flags, in_maps = _host_prep(inputs)
    if flags["ffn_bias"] and not flags["attn_bias"]:
        # mha's bo path reads attn-bias tensors; force-declare them
        flags["attn_bias"] = True
        be = np.asarray(inputs["enc_attn_b"], np.float32).copy()
        be[:, 0] *= np.float32(1.0 / math.sqrt(DK))
        bs = np.asarray(inputs["dec_sa_b"], np.float32).copy()
        bs[:, 0] *= np.float32(1.0 / math.sqrt(DK))
        bc = np.asarray(inputs["dec_ca_b"], np.float32).copy()
        bc[:, 0] *= np.float32(1.0 / math.sqrt(DK))
        for m in in_maps:
            m.update(be_attn=np.ascontiguousarray(be),
                     bd_sa=np.ascontiguousarray(bs),
                     bd_ca=np.ascontiguousarray(bc))
    nc = _build_program(flags)
    res = run_bass_kernel_spmd(nc, in_maps, list(range(NCORES)), trace=trace, **kwargs)
    out = np.zeros((4, S, V), dtype=np.float32)
    for core in range(NCORES):
        b, half = core // 2, core % 2
        out[b, :, half * VH : (half + 1) * VH] = res.results[core]["logits"]
    return out, res


def kernel(**inputs):
    out, _ = _run(inputs, trace=False)
    return out

